# revision 1
# baseline (speedup 1.0000x reference)
"""GIN encoder (3x GINConv+BN + per-layer global_add_pool) on 8 Trainium2 cores.

Strategy:
  - Nodes sharded round 8 cores (12500 each). Each core's nodes are permuted
    into 100 "tiles" of 125 seats (+3 pad) chosen to balance gather traffic
    across 4 source banks.
  - Edges sharded by dst owner; per (dst-tile, src-bank) edge slots padded to
    a fixed cap -> fully uniform SPMD program.
  - Aggregation h[src] gather via dma_gather (4 SWDGE queues) from a
    replicated [102400, 128] node-feature table (input x for layer 0,
    AllGather outputs for layers 1, 2), then segment-sum as one-hot matmuls
    accumulating in PSUM.
  - MLP feature-major on PE (weights stationary), BN stats via [128,2]
    AllReduce, pool via one-hot matmul; host sums per-core partial pools.
"""
import sys
sys.path.insert(0, '/opt/trn_rl_repo')

import numpy as np

import concourse.bass as bass
import concourse.tile as tile
from concourse import bacc, mybir, library_config
from concourse import bass_utils

# ---------------- problem config (hardcoded from spec) ----------------
NCORES = 8
N = 100000
F = 128
E = 1600000
L = 3
NUM_GRAPHS = 512
BN_EPS = 1e-5
P = 128


class Cfg:
    def __init__(self, N, E, num_graphs, T, seats, capb):
        self.N = N
        self.E = E
        self.num_graphs = num_graphs
        self.NPC = N // NCORES            # nodes per core
        self.T = T                        # dst tiles per core
        self.SEATS = seats                # real seats per tile (<=128)
        self.CAPB = capb                  # slots per (tile, bank); %128==0
        self.BCPT = capb // 128           # chunks per (tile, bank)
        self.COLS = T * P                 # padded node columns per core
        self.HF_ROWS = NCORES * self.COLS
        self.NBANK = 4
        self.BANK_ROWS = self.HF_ROWS // 4
        assert self.BANK_ROWS < 32767
        assert self.T * self.SEATS >= self.NPC
        assert self.CAPB <= 1024 and self.CAPB % 128 == 0


FULL = Cfg(N, E, NUM_GRAPHS, T=102, seats=125, capb=512)


# ---------------- host-side preprocessing ----------------
def preprocess(cfg, x, edge_index, batch):
    """Returns per-core input dicts + metadata for output assembly."""
    NPC, T, SEATS = cfg.NPC, cfg.T, cfg.SEATS
    src = edge_index[0].astype(np.int64)
    dst = edge_index[1].astype(np.int64)
    owner = dst // NPC
    src_bank = src // (2 * NPC)          # = owner(src)//2, permutation-invariant

    # --- per-core tile assignment balancing per-bank degree ---
    tile_of_g = np.empty(cfg.N, np.int64)     # local tile of each node
    seat_of_g = np.empty(cfg.N, np.int64)     # seat within tile
    capb_used = cfg.CAPB
    for c in range(NCORES):
        m = owner == c
        d_loc = dst[m] - c * NPC
        counts = np.zeros((NPC, 4), np.int64)
        np.add.at(counts, (d_loc, src_bank[m]), 1)
        deg = counts.sum(1)
        order = np.argsort(-deg, kind='stable')
        for cap_try in (cfg.CAPB, cfg.CAPB + 128, cfg.CAPB + 256, 1024):
            rem = np.full((T, 4), cap_try, np.int64)
            seats = np.full(T, SEATS, np.int64)
            tile_of = np.full(NPC, -1, np.int64)
            seat_of = np.full(NPC, -1, np.int64)
            ok_all = True
            for d in order:
                v = counts[d]
                feas = (seats > 0) & (rem >= v).all(1)
                if not feas.any():
                    ok_all = False
                    break
                slack = (rem - v).min(1).astype(np.float64)
                slack[~feas] = -1e18
                t = int(np.argmax(slack))
                tile_of[d] = t
                seat_of[d] = SEATS - seats[t]
                rem[t] -= v
                seats[t] -= 1
            if ok_all:
                capb_used = max(capb_used, cap_try)
                break
        assert ok_all, "tile assignment failed even at cap 1024"
        tile_of_g[c * NPC:(c + 1) * NPC] = tile_of
        seat_of_g[c * NPC:(c + 1) * NPC] = seat_of

    CAPB = capb_used
    BCPT = CAPB // 128
    perm_pos = tile_of_g * P + seat_of_g                     # within-core col
    perm_row = (np.arange(cfg.N) // NPC) * cfg.COLS + perm_pos  # global row

    # --- x tables ---
    xf = np.asarray(x, np.float32)
    x_full = np.zeros((cfg.HF_ROWS, F), np.float32)
    x_full[perm_row] = xf
    g_bases, spans = [], []
    per_core = []
    for c in range(NCORES):
        lo, hi = c * NPC, (c + 1) * NPC
        xlT = np.zeros((cfg.COLS, F), np.float32)
        xlT[perm_pos[lo:hi]] = xf[lo:hi]
        xlT = np.ascontiguousarray(xlT.T)                    # [128, COLS]

        g_base = int(batch[lo])
        span = int(batch[hi - 1]) - g_base + 1
        assert span <= P, f"graph span {span} exceeds 128"
        g_bases.append(g_base)
        spans.append(span)
        br = np.full(cfg.COLS, -1.0, np.float32)
        br[perm_pos[lo:hi]] = (batch[lo:hi] - g_base).astype(np.float32)
        batch_rel = np.ascontiguousarray(br.reshape(T, P).T)  # [128, T]
        filled_row = np.bincount(tile_of_g[lo:hi], minlength=T).astype(np.float32)
        filled_tbl = np.tile(filled_row, (P, 1))              # [128, T]

        # --- edge slot tables ---
        m = owner == c
        e_src = src[m]
        e_dst = dst[m] - c * NPC
        key = tile_of_g[c * NPC + e_dst] * 4 + src_bank[m]
        order_e = np.argsort(key, kind='stable')
        key_s = key[order_e]
        cnt = np.bincount(key_s, minlength=T * 4)
        assert cnt.max() <= CAPB, f"(tile,bank) count {cnt.max()} > cap {CAPB}"
        cstart = np.zeros(T * 4, np.int64)
        cstart[1:] = np.cumsum(cnt)[:-1]
        within = np.arange(len(key_s)) - np.repeat(cstart, cnt)
        slot = key_s * CAPB + within
        gidx_flat = np.zeros(T * 4 * CAPB, np.int64)
        drel_flat = np.full(T * 4 * CAPB, -1.0, np.float32)
        gidx_flat[slot] = perm_row[e_src[order_e]] % cfg.BANK_ROWS
        drel_flat[slot] = seat_of_g[c * NPC + e_dst[order_e]]
        # idx wrap: per call (t,b): CAPB values -> [16, CAPB//16], tile x8
        w = gidx_flat.reshape(T * 4, CAPB // 16, 16).transpose(2, 0, 1)
        w = w.reshape(16, T * 4 * (CAPB // 16))
        gidx_h = np.tile(w, (8, 1)).astype(np.int16)         # [128, cols]
        drel_h = np.ascontiguousarray(
            drel_flat.reshape(T * 4 * BCPT, P).T)             # [128, chunks]

        per_core.append(dict(xT=xlT, batch_rel=batch_rel,
                             gidx=gidx_h, dstrel=drel_h, filled=filled_tbl))

    meta = dict(CAPB=CAPB, BCPT=BCPT, g_bases=g_bases, spans=spans)
    return x_full, per_core, meta


# ---------------- device kernel builder ----------------
def build_kernel(cfg, BCPT, num_swdge_queues=4, repeat=1, loop_n=1, no_cc=False, no_gather=False):
    CAPB = BCPT * 128
    T = cfg.T
    dt = mybir.dt
    nc = bacc.Bacc("TRN2", target_bir_lowering=False, debug=False,
                   enable_asserts=True, num_devices=NCORES,
                   num_swdge_queues=num_swdge_queues)

    # inputs
    xT_d = nc.dram_tensor("xT", [P, cfg.COLS], dt.float32, kind="ExternalInput")
    xfull_d = nc.dram_tensor("xfull", [cfg.HF_ROWS, F], dt.float32,
                             kind="ExternalInput")
    gidx_d = nc.dram_tensor("gidx", [P, T * 4 * (CAPB // 16)], dt.int16,
                            kind="ExternalInput")
    dstrel_d = nc.dram_tensor("dstrel", [P, T * 4 * BCPT], dt.float32,
                              kind="ExternalInput")
    brel_d = nc.dram_tensor("brel", [P, T], dt.float32, kind="ExternalInput")
    filled_d = nc.dram_tensor("filled", [P, T], dt.float32, kind="ExternalInput")
    w1_d = nc.dram_tensor("w1", [P, L * F], dt.float32, kind="ExternalInput")
    w2_d = nc.dram_tensor("w2", [P, L * F], dt.float32, kind="ExternalInput")
    bias_d = nc.dram_tensor("bias", [P, 4 * L], dt.float32, kind="ExternalInput")
    iota_d = nc.dram_tensor("iotat", [P, P], dt.float32, kind="ExternalInput")
    ident_d = nc.dram_tensor("ident", [P, P], dt.float32, kind="ExternalInput")
    pools_d = nc.dram_tensor("pools", [P, L * F], dt.float32,
                             kind="ExternalOutput")

    # internal DRAM
    zz = [nc.dram_tensor(f"zz{l}", [cfg.COLS, F], dt.float32, kind="Internal")
          for l in range(L - 1)]
    hf = [nc.dram_tensor(f"hf{l}", [cfg.HF_ROWS, F], dt.float32,
                         kind="Internal", addr_space="Shared")
          for l in range(L - 1)]
    st_in = [nc.dram_tensor(f"stin{l}", [P, 2], dt.float32, kind="Internal")
             for l in range(L)]
    st_out = [nc.dram_tensor(f"stout{l}", [P, 2], dt.float32, kind="Internal",
                             addr_space="Shared") for l in range(L)]

    inv_n = 1.0 / cfg.N
    groups = [list(range(NCORES))]

    with tile.TileContext(nc) as tc:
        with tc.tile_pool(name="big", bufs=1) as big, \
             tc.tile_pool(name="gpool", bufs=3) as gpool, \
             tc.tile_pool(name="spool", bufs=8) as spool, \
             tc.tile_pool(name="work", bufs=4) as work, \
             tc.tile_pool(name="stat", bufs=1) as statp, \
             tc.tile_pool(name="psA", bufs=2, space="PSUM") as psA, \
             tc.tile_pool(name="psT", bufs=2, space="PSUM") as psT, \
             tc.tile_pool(name="psM", bufs=2, space="PSUM") as psM, \
             tc.tile_pool(name="psP", bufs=1, space="PSUM") as psP:

            nc.gpsimd.load_library(library_config.mlp)

            h_loc = big.tile([P, cfg.COLS], dt.float32)       # feature-major h
            nc.sync.dma_start(h_loc[:], xT_d.ap())
            gidx_t = big.tile([P, T * 4 * (CAPB // 16)], dt.int16)
            nc.sync.dma_start(gidx_t[:], gidx_d.ap())
            drel_t = big.tile([P, T * 4 * BCPT], dt.float32)
            nc.sync.dma_start(drel_t[:], dstrel_d.ap())
            brel_t = big.tile([P, T], dt.float32)
            nc.sync.dma_start(brel_t[:], brel_d.ap())
            filled_t = big.tile([P, T], dt.float32)
            nc.sync.dma_start(filled_t[:], filled_d.ap())
            w1_t = big.tile([P, L * F], dt.float32)
            nc.sync.dma_start(w1_t[:], w1_d.ap())
            w2_t = big.tile([P, L * F], dt.float32)
            nc.sync.dma_start(w2_t[:], w2_d.ap())
            bias_t = big.tile([P, 4 * L], dt.float32)
            nc.sync.dma_start(bias_t[:], bias_d.ap())
            iota_t = big.tile([P, P], dt.float32)
            nc.sync.dma_start(iota_t[:], iota_d.ap())
            ident_t = big.tile([P, P], dt.float32)
            nc.sync.dma_start(ident_t[:], ident_d.ap())

            from contextlib import nullcontext
            with (tc.For_i(0, loop_n, 1) if loop_n > 1 else nullcontext()):
             for rep in range(repeat):
              for l in range(L):
                hsrc = xfull_d if (l == 0 or no_cc) else hf[l - 1]
                b1c = bias_t[:, 0 * L + l:0 * L + l + 1]
                b2c = bias_t[:, 1 * L + l:1 * L + l + 1]
                gac = bias_t[:, 2 * L + l:2 * L + l + 1]
                bec = bias_t[:, 3 * L + l:3 * L + l + 1]
                w1c = w1_t[:, l * F:(l + 1) * F]
                w2c = w2_t[:, l * F:(l + 1) * F]

                ssum = statp.tile([P, T], dt.float32, tag=f"ssum{l}")
                ssq = statp.tile([P, T], dt.float32, tag=f"ssq{l}")

                for t in range(T):
                    g_t = gpool.tile([P, 4 * BCPT, P], dt.float32, tag="G")
                    if no_gather:
                        nc.scalar.copy(g_t[:, 0, :], iota_t[:])
                    for b in range(4 if not no_gather else 0):
                        call = t * 4 + b
                        nc.gpsimd.dma_gather(
                            out_ap=g_t[:, b * BCPT:(b + 1) * BCPT, :],
                            in_ap=hsrc.ap()[b * cfg.BANK_ROWS:
                                            (b + 1) * cfg.BANK_ROWS, :],
                            idxs_ap=gidx_t[:, call * (CAPB // 16):
                                           (call + 1) * (CAPB // 16)],
                            num_idxs=CAPB,
                            num_idxs_reg=CAPB,
                            elem_size=F,
                            queue_num=b % num_swdge_queues,
                        )
                    agg_ps = psA.tile([P, P], dt.float32, tag="agg")
                    nch = 4 * BCPT
                    for ch in range(nch):
                        s_t = spool.tile([P, P], dt.float32, tag="S")
                        nc.vector.tensor_scalar(
                            out=s_t[:], in0=iota_t[:],
                            scalar1=drel_t[:, t * nch + ch:t * nch + ch + 1],
                            scalar2=None, op0=mybir.AluOpType.is_equal)
                        nc.tensor.matmul(agg_ps[:], lhsT=s_t[:],
                                         rhs=g_t[:, 0 if no_gather else ch, :],
                                         start=(ch == 0), stop=(ch == nch - 1))
                    # node-major agg -> SBUF -> transpose to feature-major
                    agg_nm = work.tile([P, P], dt.float32, tag="aggnm")
                    nc.scalar.copy(agg_nm[:], agg_ps[:])
                    aggT_ps = psT.tile([P, P], dt.float32, tag="aggT")
                    nc.tensor.transpose(aggT_ps[:], agg_nm[:], ident_t[:])
                    z1in = work.tile([P, P], dt.float32, tag="z1in")
                    nc.vector.tensor_add(z1in[:], h_loc[:, t * P:(t + 1) * P],
                                         aggT_ps[:])
                    # MLP (feature-major, weights stationary)
                    mp1 = psM.tile([P, P], dt.float32, tag="mp")
                    nc.tensor.matmul(mp1[:], lhsT=w1c, rhs=z1in[:],
                                     start=True, stop=True)
                    z1 = work.tile([P, P], dt.float32, tag="z1")
                    nc.scalar.activation(z1[:], mp1[:],
                                         mybir.ActivationFunctionType.Relu,
                                         bias=b1c)
                    mp2 = psM.tile([P, P], dt.float32, tag="mp")
                    nc.tensor.matmul(mp2[:], lhsT=w2c, rhs=z1[:],
                                     start=True, stop=True)
                    # z_pre overwrites h_loc tile in place
                    nc.scalar.activation(h_loc[:, t * P:(t + 1) * P], mp2[:],
                                         mybir.ActivationFunctionType.Relu,
                                         bias=b2c)
                    # zero phantom (unfilled + pad) seat columns, then stats
                    msk = spool.tile([P, P], dt.float32, tag="S")
                    nc.vector.tensor_scalar(
                        out=msk[:], in0=iota_t[:],
                        scalar1=filled_t[:, t:t + 1], scalar2=None,
                        op0=mybir.AluOpType.is_lt)
                    nc.vector.tensor_tensor(
                        out=h_loc[:, t * P:(t + 1) * P],
                        in0=h_loc[:, t * P:(t + 1) * P], in1=msk[:],
                        op=mybir.AluOpType.mult)
                    seat_ap = h_loc[:, t * P:(t + 1) * P]
                    nc.vector.tensor_reduce(
                        out=ssum[:, t:t + 1], in_=seat_ap,
                        axis=mybir.AxisListType.X, op=mybir.AluOpType.add)
                    sqs = work.tile([P, P], dt.float32, tag="sqs")
                    nc.scalar.activation(sqs[:], seat_ap,
                                         mybir.ActivationFunctionType.Square,
                                         accum_out=ssq[:, t:t + 1])

                # ---- BN stats allreduce ----
                red = work.tile([P, 2], dt.float32, tag="red")
                nc.vector.tensor_reduce(out=red[:, 0:1], in_=ssum[:],
                                        axis=mybir.AxisListType.X,
                                        op=mybir.AluOpType.add)
                nc.vector.tensor_reduce(out=red[:, 1:2], in_=ssq[:],
                                        axis=mybir.AxisListType.X,
                                        op=mybir.AluOpType.add)
                nc.sync.dma_start(st_in[l].ap(), red[:])
                if not no_cc:
                    nc.gpsimd.collective_compute(
                        "AllReduce", mybir.AluOpType.add, replica_groups=groups,
                        ins=[st_in[l].ap().opt()], outs=[st_out[l].ap().opt()])
                stt = work.tile([P, 2], dt.float32, tag="stt")
                nc.sync.dma_start(stt[:], (st_in[l] if no_cc else st_out[l]).ap())
                # mean, var, scale a, shift c
                mean = work.tile([P, 1], dt.float32, tag="mean")
                nc.vector.tensor_scalar(out=mean[:], in0=stt[:, 0:1],
                                        scalar1=inv_n, scalar2=None,
                                        op0=mybir.AluOpType.mult)
                var = work.tile([P, 1], dt.float32, tag="var")
                nc.vector.tensor_scalar(out=var[:], in0=stt[:, 1:2],
                                        scalar1=inv_n, scalar2=None,
                                        op0=mybir.AluOpType.mult)
                msq = work.tile([P, 1], dt.float32, tag="msq")
                nc.vector.tensor_tensor(out=msq[:], in0=mean[:], in1=mean[:],
                                        op=mybir.AluOpType.mult)
                nc.vector.tensor_tensor(out=var[:], in0=var[:], in1=msq[:],
                                        op=mybir.AluOpType.subtract)
                nc.vector.tensor_scalar(out=var[:], in0=var[:],
                                        scalar1=BN_EPS, scalar2=None,
                                        op0=mybir.AluOpType.add)
                sd = work.tile([P, 1], dt.float32, tag="sd")
                nc.scalar.activation(sd[:], var[:],
                                     mybir.ActivationFunctionType.Sqrt)
                inv = work.tile([P, 1], dt.float32, tag="inv")
                nc.vector.reciprocal(inv[:], sd[:])
                a_t = work.tile([P, 1], dt.float32, tag="a")
                nc.vector.tensor_tensor(out=a_t[:], in0=inv[:], in1=gac,
                                        op=mybir.AluOpType.mult)
                c_t = work.tile([P, 1], dt.float32, tag="c")
                nc.vector.tensor_tensor(out=c_t[:], in0=mean[:], in1=a_t[:],
                                        op=mybir.AluOpType.mult)
                nc.vector.tensor_tensor(out=c_t[:], in0=bec, in1=c_t[:],
                                        op=mybir.AluOpType.subtract)

                # ---- normalize in place, transpose, pool, store ----
                pool_ps = psP.tile([P, P], dt.float32, tag="pool")
                for t in range(T):
                    nc.scalar.activation(
                        h_loc[:, t * P:(t + 1) * P],
                        h_loc[:, t * P:(t + 1) * P],
                        mybir.ActivationFunctionType.Identity,
                        bias=c_t[:, 0:1], scale=a_t[:, 0:1])
                    zT_ps = psT.tile([P, P], dt.float32, tag="aggT")
                    nc.tensor.transpose(zT_ps[:], h_loc[:, t * P:(t + 1) * P],
                                        ident_t[:])
                    znm = work.tile([P, P], dt.float32, tag="znm")
                    nc.scalar.copy(znm[:], zT_ps[:])
                    if l < L - 1:
                        nc.sync.dma_start(zz[l].ap()[t * P:(t + 1) * P, :],
                                          znm[:])
                    sb_t = spool.tile([P, P], dt.float32, tag="S")
                    nc.vector.tensor_scalar(
                        out=sb_t[:], in0=iota_t[:],
                        scalar1=brel_t[:, t:t + 1], scalar2=None,
                        op0=mybir.AluOpType.is_equal)
                    nc.tensor.matmul(pool_ps[:], lhsT=sb_t[:], rhs=znm[:],
                                     start=(t == 0), stop=(t == T - 1),
                                     skip_group_check=True)
                poolsb = work.tile([P, P], dt.float32, tag="poolsb")
                nc.scalar.copy(poolsb[:], pool_ps[:])
                nc.sync.dma_start(pools_d.ap()[:, l * F:(l + 1) * F],
                                  poolsb[:])

                if l < L - 1 and not no_cc:
                    nc.gpsimd.collective_compute(
                        "AllGather", mybir.AluOpType.bypass,
                        replica_groups=groups,
                        ins=[zz[l].ap().opt()], outs=[hf[l].ap().opt()])

    nc.compile()
    return nc


def make_in_maps(per_core, x_full, inputs):
    W1 = np.asarray(inputs['W1'], np.float32)
    W2 = np.asarray(inputs['W2'], np.float32)
    b1 = np.asarray(inputs['b1'], np.float32)
    b2 = np.asarray(inputs['b2'], np.float32)
    gamma = np.asarray(inputs['gamma'], np.float32)
    beta = np.asarray(inputs['beta'], np.float32)
    w1_h = np.ascontiguousarray(np.concatenate([W1[i] for i in range(L)], 1))
    w2_h = np.ascontiguousarray(np.concatenate([W2[i] for i in range(L)], 1))
    bias_h = np.ascontiguousarray(
        np.concatenate([b1.T, b2.T, gamma.T, beta.T], 1))
    iota_h = np.tile(np.arange(P, dtype=np.float32), (P, 1))
    ident_h = np.eye(P, dtype=np.float32)
    in_maps = []
    for c in range(NCORES):
        pc = per_core[c]
        in_maps.append({
            "xT": pc["xT"], "xfull": x_full, "gidx": pc["gidx"],
            "dstrel": pc["dstrel"], "brel": pc["batch_rel"],
            "filled": pc["filled"],
            "w1": w1_h, "w2": w2_h, "bias": bias_h,
            "iotat": iota_h, "ident": ident_h,
        })
    return in_maps


def build_null_kernel(cfg, BCPT):
    """Same I/O signature as the real kernel, trivial body. Used to measure
    the non-compute overhead (upload/dispatch) of a run for timing deltas."""
    CAPB = BCPT * 128
    T = cfg.T
    dt = mybir.dt
    nc = bacc.Bacc("TRN2", target_bir_lowering=False, debug=False,
                   enable_asserts=True, num_devices=NCORES,
                   num_swdge_queues=4)
    nc.dram_tensor("xT", [P, cfg.COLS], dt.float32, kind="ExternalInput")
    xfull_d = nc.dram_tensor("xfull", [cfg.HF_ROWS, F], dt.float32,
                             kind="ExternalInput")
    nc.dram_tensor("gidx", [P, T * 4 * (CAPB // 16)], dt.int16,
                   kind="ExternalInput")
    nc.dram_tensor("dstrel", [P, T * 4 * BCPT], dt.float32,
                   kind="ExternalInput")
    nc.dram_tensor("brel", [P, T], dt.float32, kind="ExternalInput")
    nc.dram_tensor("filled", [P, T], dt.float32, kind="ExternalInput")
    nc.dram_tensor("w1", [P, L * F], dt.float32, kind="ExternalInput")
    nc.dram_tensor("w2", [P, L * F], dt.float32, kind="ExternalInput")
    nc.dram_tensor("bias", [P, 4 * L], dt.float32, kind="ExternalInput")
    nc.dram_tensor("iotat", [P, P], dt.float32, kind="ExternalInput")
    nc.dram_tensor("ident", [P, P], dt.float32, kind="ExternalInput")
    pools_d = nc.dram_tensor("pools", [P, L * F], dt.float32,
                             kind="ExternalOutput")
    with tile.TileContext(nc) as tc:
        with tc.tile_pool(name="sb", bufs=1) as sb:
            t0 = sb.tile([P, L, F], dt.float32)
            nc.sync.dma_start(
                t0[:], xfull_d.ap()[0:L * P, :]
                .rearrange("(p a) f -> p a f", p=P))
            nc.sync.dma_start(pools_d.ap(),
                              t0[:].rearrange("p a f -> p (a f)"))
    nc.compile()
    return nc


# ---------------- entry point ----------------
def kernel(x, edge_index, batch, W1, b1, W2, b2, gamma, beta):
    cfg = FULL
    x = np.asarray(x, np.float32)
    edge_index = np.asarray(edge_index, np.int32)
    batch = np.asarray(batch, np.int32)
    W1 = np.asarray(W1, np.float32)
    b1 = np.asarray(b1, np.float32)
    W2 = np.asarray(W2, np.float32)
    b2 = np.asarray(b2, np.float32)
    gamma = np.asarray(gamma, np.float32)
    beta = np.asarray(beta, np.float32)

    x_full, per_core, meta = preprocess(cfg, x, edge_index, batch)
    nc = build_kernel(cfg, meta['BCPT'])
    in_maps = make_in_maps(per_core, x_full, dict(
        W1=W1, W2=W2, b1=b1, b2=b2, gamma=gamma, beta=beta))

    import time as _time
    last_exc = None
    for attempt in range(3):
        try:
            res = bass_utils.run_bass_kernel_spmd(
                nc, in_maps, core_ids=list(range(NCORES)))
            break
        except Exception as e:       # transient device wedge -> retry
            last_exc = e
            _time.sleep(20)
    else:
        raise last_exc

    out = np.zeros((NUM_GRAPHS, L * F), np.float32)
    for c in range(NCORES):
        pool_c = res.results[c]["pools"]          # [128, L*F]
        g0, sp = meta['g_bases'][c], meta['spans'][c]
        out[g0:g0 + sp] += pool_c[:sp]
    return out


if __name__ == "__main__":
    import reference
    inputs = reference.setup_inputs()
    inputs = {k: np.asarray(v) for k, v in inputs.items()}
    got = kernel(**inputs)
    print("kernel output shape:", got.shape)



# revision 9
# speedup vs baseline: 34.9814x; 34.9814x over previous
"""GIN encoder (3x GINConv+BN + per-layer global_add_pool) on 8 Trainium2 cores.

Strategy:
  - Nodes sharded round 8 cores (12500 each). Each core's nodes are permuted
    into 100 "tiles" of 125 seats (+3 pad) chosen to balance gather traffic
    across 4 source banks.
  - Edges sharded by dst owner; per (dst-tile, src-bank) edge slots padded to
    a fixed cap -> fully uniform SPMD program.
  - Aggregation h[src] gather via dma_gather (4 SWDGE queues) from a
    replicated [102400, 128] node-feature table (input x for layer 0,
    AllGather outputs for layers 1, 2), then segment-sum as one-hot matmuls
    accumulating in PSUM.
  - MLP feature-major on PE (weights stationary), BN stats via [128,2]
    AllReduce, pool via one-hot matmul; host sums per-core partial pools.
"""
import sys
sys.path.insert(0, '/opt/trn_rl_repo')

import numpy as np

import concourse.bass as bass
import concourse.tile as tile
from concourse import bacc, mybir, library_config
from concourse import bass_utils

# ---------------- problem config (hardcoded from spec) ----------------
NCORES = 8
N = 100000
F = 128
E = 1600000
L = 3
NUM_GRAPHS = 512
BN_EPS = 1e-5
P = 128


class Cfg:
    def __init__(self, N, E, num_graphs, T, seats, capb):
        self.N = N
        self.E = E
        self.num_graphs = num_graphs
        self.NPC = N // NCORES            # nodes per core
        self.T = T                        # dst tiles per core
        self.SEATS = seats                # real seats per tile (<=128)
        self.CAPB = capb                  # slots per (tile, bank); %128==0
        self.BCPT = capb // 128           # chunks per (tile, bank)
        self.COLS = T * P                 # padded node columns per core
        self.HF_ROWS = NCORES * self.COLS
        self.NBANK = 4
        self.BANK_ROWS = self.HF_ROWS // 4
        assert self.BANK_ROWS < 32767
        assert self.T * self.SEATS >= self.NPC
        assert self.CAPB <= 1024 and self.CAPB % 128 == 0


FULL = Cfg(N, E, NUM_GRAPHS, T=102, seats=125, capb=512)


# ---------------- host-side preprocessing ----------------
def preprocess(cfg, x, edge_index, batch):
    """Returns per-core input dicts + metadata for output assembly."""
    NPC, T, SEATS = cfg.NPC, cfg.T, cfg.SEATS
    src = edge_index[0].astype(np.int64)
    dst = edge_index[1].astype(np.int64)
    owner = dst // NPC
    src_bank = src // (2 * NPC)          # = owner(src)//2, permutation-invariant

    # --- per-core tile assignment balancing per-bank degree ---
    tile_of_g = np.empty(cfg.N, np.int64)     # local tile of each node
    seat_of_g = np.empty(cfg.N, np.int64)     # seat within tile
    capb_used = cfg.CAPB
    for c in range(NCORES):
        m = owner == c
        d_loc = dst[m] - c * NPC
        counts = np.zeros((NPC, 4), np.int64)
        np.add.at(counts, (d_loc, src_bank[m]), 1)
        deg = counts.sum(1)
        order = np.argsort(-deg, kind='stable')
        for cap_try in (cfg.CAPB, cfg.CAPB + 128, cfg.CAPB + 256, 1024):
            rem = np.full((T, 4), cap_try, np.int64)
            seats = np.full(T, SEATS, np.int64)
            tile_of = np.full(NPC, -1, np.int64)
            seat_of = np.full(NPC, -1, np.int64)
            ok_all = True
            for d in order:
                v = counts[d]
                feas = (seats > 0) & (rem >= v).all(1)
                if not feas.any():
                    ok_all = False
                    break
                slack = (rem - v).min(1).astype(np.float64)
                slack[~feas] = -1e18
                t = int(np.argmax(slack))
                tile_of[d] = t
                seat_of[d] = SEATS - seats[t]
                rem[t] -= v
                seats[t] -= 1
            if ok_all:
                capb_used = max(capb_used, cap_try)
                break
        assert ok_all, "tile assignment failed even at cap 1024"
        tile_of_g[c * NPC:(c + 1) * NPC] = tile_of
        seat_of_g[c * NPC:(c + 1) * NPC] = seat_of

    CAPB = capb_used
    BCPT = CAPB // 128
    perm_pos = tile_of_g * P + seat_of_g                     # within-core col
    perm_row = (np.arange(cfg.N) // NPC) * cfg.COLS + perm_pos  # global row

    # --- x tables ---
    xf = np.asarray(x, np.float32)
    x_full = np.zeros((cfg.HF_ROWS, F), np.float32)
    x_full[perm_row] = xf
    g_bases, spans = [], []
    per_core = []
    for c in range(NCORES):
        lo, hi = c * NPC, (c + 1) * NPC
        xlT = np.zeros((cfg.COLS, F), np.float32)
        xlT[perm_pos[lo:hi]] = xf[lo:hi]
        xlT = np.ascontiguousarray(xlT.T)                    # [128, COLS]

        g_base = int(batch[lo])
        span = int(batch[hi - 1]) - g_base + 1
        assert span <= P, f"graph span {span} exceeds 128"
        g_bases.append(g_base)
        spans.append(span)
        br = np.full(cfg.COLS, -1.0, np.float32)
        br[perm_pos[lo:hi]] = (batch[lo:hi] - g_base).astype(np.float32)
        batch_rel = np.ascontiguousarray(br.reshape(T, P).T)  # [128, T]
        filled_row = np.bincount(tile_of_g[lo:hi], minlength=T).astype(np.float32)
        filled_tbl = np.tile(filled_row, (P, 1))              # [128, T]

        # --- edge slot tables ---
        m = owner == c
        e_src = src[m]
        e_dst = dst[m] - c * NPC
        key = tile_of_g[c * NPC + e_dst] * 4 + src_bank[m]
        order_e = np.argsort(key, kind='stable')
        key_s = key[order_e]
        cnt = np.bincount(key_s, minlength=T * 4)
        assert cnt.max() <= CAPB, f"(tile,bank) count {cnt.max()} > cap {CAPB}"
        cstart = np.zeros(T * 4, np.int64)
        cstart[1:] = np.cumsum(cnt)[:-1]
        within = np.arange(len(key_s)) - np.repeat(cstart, cnt)
        slot = key_s * CAPB + within
        gidx_flat = np.zeros(T * 4 * CAPB, np.int64)
        drel_flat = np.full(T * 4 * CAPB, -1.0, np.float32)
        gidx_flat[slot] = perm_row[e_src[order_e]] % cfg.BANK_ROWS
        drel_flat[slot] = seat_of_g[c * NPC + e_dst[order_e]]
        # idx wrap: per call (t,b): CAPB values -> [16, CAPB//16], tile x8
        w = gidx_flat.reshape(T * 4, CAPB // 16, 16).transpose(2, 0, 1)
        w = w.reshape(16, T * 4 * (CAPB // 16))
        gidx_h = np.tile(w, (8, 1)).astype(np.int16)         # [128, cols]
        drel_h = np.ascontiguousarray(
            drel_flat.reshape(T * 4 * BCPT, P).T)             # [128, chunks]

        per_core.append(dict(xT=xlT, batch_rel=batch_rel,
                             gidx=gidx_h, dstrel=drel_h, filled=filled_tbl))

    meta = dict(CAPB=CAPB, BCPT=BCPT, g_bases=g_bases, spans=spans)
    return x_full, per_core, meta


# ---------------- device kernel builder (v2: hardware loops) ----------------
def build_kernel_v2(cfg, BCPT, num_swdge_queues=4, repeat=1, no_cc=False,
                    fat_onehot=True):
    """Same algorithm as v1 but the per-tile pipelines run inside For_i
    hardware loops with register-indexed APs. Static instruction count
    ~220 vs ~15.6k; this runtime charges ~127us per STATIC instruction,
    so this is the dominant win."""
    from concourse.bass import ds
    CAPB = BCPT * 128
    T = cfg.T
    nch = 4 * BCPT
    C16 = CAPB // 16
    dt = mybir.dt
    nc = bacc.Bacc("TRN2", target_bir_lowering=False, debug=False,
                   enable_asserts=True, num_devices=NCORES,
                   num_swdge_queues=num_swdge_queues)

    # inputs (v1 signature minus the replicated xfull table: the layer-0
    # gather table is AllGathered on device from the per-core xT slices)
    xT_d = nc.dram_tensor("xT", [P, cfg.COLS], dt.float32, kind="ExternalInput")
    gidx_d = nc.dram_tensor("gidx", [P, T * 4 * C16], dt.int16,
                            kind="ExternalInput")
    dstrel_d = nc.dram_tensor("dstrel", [P, T * nch], dt.float32,
                              kind="ExternalInput")
    xnm_d = nc.dram_tensor("xnm", [cfg.COLS, F], dt.float32, kind="Internal")
    xg_d = nc.dram_tensor("xg", [cfg.HF_ROWS, F], dt.float32,
                          kind="Internal", addr_space="Shared")
    brel_d = nc.dram_tensor("brel", [P, T], dt.float32, kind="ExternalInput")
    filled_d = nc.dram_tensor("filled", [P, T], dt.float32, kind="ExternalInput")
    w1_d = nc.dram_tensor("w1", [P, L * F], dt.float32, kind="ExternalInput")
    w2_d = nc.dram_tensor("w2", [P, L * F], dt.float32, kind="ExternalInput")
    bias_d = nc.dram_tensor("bias", [P, 4 * L], dt.float32, kind="ExternalInput")
    iota_d = nc.dram_tensor("iotat", [P, P], dt.float32, kind="ExternalInput")
    ident_d = nc.dram_tensor("ident", [P, P], dt.float32, kind="ExternalInput")
    pools_d = nc.dram_tensor("pools", [P, L * F], dt.float32,
                             kind="ExternalOutput")

    zz = [nc.dram_tensor(f"zz{l}", [cfg.COLS, F], dt.float32, kind="Internal")
          for l in range(L - 1)]
    hf = [nc.dram_tensor(f"hf{l}", [cfg.HF_ROWS, F], dt.float32,
                         kind="Internal", addr_space="Shared")
          for l in range(L - 1)]
    st_in = [nc.dram_tensor(f"stin{l}", [P, 2], dt.float32, kind="Internal")
             for l in range(L)]
    st_out = [nc.dram_tensor(f"stout{l}", [P, 2], dt.float32, kind="Internal",
                             addr_space="Shared") for l in range(L)]

    inv_n = 1.0 / cfg.N
    groups = [list(range(NCORES))]

    with tile.TileContext(nc) as tc:
        with tc.tile_pool(name="big", bufs=1) as big, \
             tc.tile_pool(name="gpool", bufs=2) as gpool, \
             tc.tile_pool(name="spool", bufs=2) as spool, \
             tc.tile_pool(name="work", bufs=2) as work, \
             tc.tile_pool(name="stat", bufs=1) as statp, \
             tc.tile_pool(name="psA", bufs=2, space="PSUM") as psA, \
             tc.tile_pool(name="psT", bufs=2, space="PSUM") as psT, \
             tc.tile_pool(name="psM", bufs=2, space="PSUM") as psM, \
             tc.tile_pool(name="psP", bufs=2, space="PSUM") as psP:

            nc.gpsimd.load_library(library_config.mlp)

            h_loc = big.tile([P, cfg.COLS], dt.float32)
            nc.sync.dma_start(h_loc[:], xT_d.ap())
            gidx_t = big.tile([P, T * 4 * C16], dt.int16)
            nc.sync.dma_start(gidx_t[:], gidx_d.ap())
            drel_t = big.tile([P, T * nch], dt.float32)
            nc.sync.dma_start(drel_t[:], dstrel_d.ap())
            brel_t = big.tile([P, T], dt.float32)
            nc.sync.dma_start(brel_t[:], brel_d.ap())
            filled_t = big.tile([P, T], dt.float32)
            nc.sync.dma_start(filled_t[:], filled_d.ap())
            w1_t = big.tile([P, L * F], dt.float32)
            nc.sync.dma_start(w1_t[:], w1_d.ap())
            w2_t = big.tile([P, L * F], dt.float32)
            nc.sync.dma_start(w2_t[:], w2_d.ap())
            bias_t = big.tile([P, 4 * L], dt.float32)
            nc.sync.dma_start(bias_t[:], bias_d.ap())
            iota_t = big.tile([P, P], dt.float32)
            nc.sync.dma_start(iota_t[:], iota_d.ap())
            ident_t = big.tile([P, P], dt.float32)
            nc.sync.dma_start(ident_t[:], ident_d.ap())

            # build the layer-0 gather table on device: transpose the local
            # feature-major x into node-major rows, AllGather across cores
            with tc.For_i(0, T, 1) as t:
                xf_s = work.tile([P, P], dt.float32, tag="znf")
                nc.scalar.copy(xf_s[:], h_loc[:, ds(t * P, P)])
                xT_ps = psT.tile([P, P], dt.float32, tag="zT")
                nc.tensor.transpose(xT_ps[:], xf_s[:], ident_t[:])
                x_nm = work.tile([P, P], dt.float32, tag="znm")
                nc.scalar.copy(x_nm[:], xT_ps[:])
                nc.sync.dma_start(xnm_d.ap()[ds(t * P, P), :], x_nm[:])
            if not no_cc:
                nc.gpsimd.collective_compute(
                    "AllGather", mybir.AluOpType.bypass,
                    replica_groups=groups,
                    ins=[xnm_d.ap().opt()], outs=[xg_d.ap().opt()])

            for rep in range(repeat):
             for l in range(L):
                hsrc = xg_d if (l == 0 or no_cc) else hf[l - 1]
                b1c = bias_t[:, 0 * L + l:0 * L + l + 1]
                b2c = bias_t[:, 1 * L + l:1 * L + l + 1]
                gac = bias_t[:, 2 * L + l:2 * L + l + 1]
                bec = bias_t[:, 3 * L + l:3 * L + l + 1]
                w1c = w1_t[:, l * F:(l + 1) * F]
                w2c = w2_t[:, l * F:(l + 1) * F]

                acc = statp.tile([P, 2], dt.float32, tag=f"acc{l}{rep}")
                nc.vector.memset(acc[:], 0.0)

                # ---- phase A: aggregate + MLP, HW loop over dst tiles ----
                with tc.For_i(0, T, 1) as t:
                    g_t = gpool.tile([P, nch, P], dt.float32, tag="G")
                    for b in range(4):
                        nc.gpsimd.dma_gather(
                            out_ap=g_t[:, b * BCPT:(b + 1) * BCPT, :],
                            in_ap=hsrc.ap()[b * cfg.BANK_ROWS:
                                            (b + 1) * cfg.BANK_ROWS, :],
                            idxs_ap=gidx_t[:, ds(t * (4 * C16) + b * C16, C16)],
                            num_idxs=CAPB,
                            num_idxs_reg=CAPB,
                            elem_size=F,
                            queue_num=b % num_swdge_queues,
                        )
                    # one-hot seat matrices for all chunks
                    if fat_onehot:
                        s_all = spool.tile([P, nch, P], dt.float32, tag="S")
                        nc.vector.tensor_tensor(
                            out=s_all[:],
                            in0=iota_t[:].unsqueeze(1)
                                .broadcast_to([P, nch, P]),
                            in1=drel_t[:, ds(t * nch, nch)].unsqueeze(2)
                                .broadcast_to([P, nch, P]),
                            op=mybir.AluOpType.is_equal)
                    else:
                        s_all = spool.tile([P, nch, P], dt.float32, tag="S")
                        for ch in range(nch):
                            nc.vector.tensor_scalar(
                                out=s_all[:, ch, :], in0=iota_t[:],
                                scalar1=drel_t[:, ds(t * nch + ch, 1)],
                                scalar2=None, op0=mybir.AluOpType.is_equal)
                    # feature-major agg: agg[f, seat] += g[e,f]^T @ onehot[e,seat]
                    agg_ps = psA.tile([P, P], dt.float32, tag="agg")
                    for ch in range(nch):
                        nc.tensor.matmul(agg_ps[:], lhsT=g_t[:, ch, :],
                                         rhs=s_all[:, ch, :],
                                         start=(ch == 0), stop=(ch == nch - 1))
                    z1in = work.tile([P, P], dt.float32, tag="z1in")
                    nc.vector.tensor_add(z1in[:], h_loc[:, ds(t * P, P)],
                                         agg_ps[:])
                    mp1 = psM.tile([P, P], dt.float32, tag="mp")
                    nc.tensor.matmul(mp1[:], lhsT=w1c, rhs=z1in[:],
                                     start=True, stop=True)
                    z1 = work.tile([P, P], dt.float32, tag="z1")
                    nc.scalar.activation(z1[:], mp1[:],
                                         mybir.ActivationFunctionType.Relu,
                                         bias=b1c)
                    mp2 = psM.tile([P, P], dt.float32, tag="mp")
                    nc.tensor.matmul(mp2[:], lhsT=w2c, rhs=z1[:],
                                     start=True, stop=True)
                    nc.scalar.activation(h_loc[:, ds(t * P, P)], mp2[:],
                                         mybir.ActivationFunctionType.Relu,
                                         bias=b2c)
                    msk = work.tile([P, P], dt.float32, tag="msk")
                    nc.vector.tensor_scalar(
                        out=msk[:], in0=iota_t[:],
                        scalar1=filled_t[:, ds(t, 1)], scalar2=None,
                        op0=mybir.AluOpType.is_lt)
                    nc.vector.tensor_tensor(
                        out=h_loc[:, ds(t * P, P)],
                        in0=h_loc[:, ds(t * P, P)], in1=msk[:],
                        op=mybir.AluOpType.mult)
                    s1 = work.tile([P, 1], dt.float32, tag="s1")
                    nc.vector.tensor_reduce(
                        out=s1[:], in_=h_loc[:, ds(t * P, P)],
                        axis=mybir.AxisListType.X, op=mybir.AluOpType.add)
                    nc.vector.tensor_tensor(out=acc[:, 0:1], in0=acc[:, 0:1],
                                            in1=s1[:],
                                            op=mybir.AluOpType.add)
                    sqs = work.tile([P, P], dt.float32, tag="sqs")
                    s2 = work.tile([P, 1], dt.float32, tag="s2")
                    nc.scalar.activation(sqs[:], h_loc[:, ds(t * P, P)],
                                         mybir.ActivationFunctionType.Square,
                                         accum_out=s2[:])
                    nc.vector.tensor_tensor(out=acc[:, 1:2], in0=acc[:, 1:2],
                                            in1=s2[:],
                                            op=mybir.AluOpType.add)

                # ---- BN stats allreduce ----
                nc.sync.dma_start(st_in[l].ap(), acc[:])
                if not no_cc:
                    nc.gpsimd.collective_compute(
                        "AllReduce", mybir.AluOpType.add, replica_groups=groups,
                        ins=[st_in[l].ap().opt()], outs=[st_out[l].ap().opt()])
                stt = work.tile([P, 2], dt.float32, tag="stt")
                nc.sync.dma_start(stt[:], (st_in[l] if no_cc else st_out[l]).ap())
                mean = work.tile([P, 1], dt.float32, tag="mean")
                nc.vector.tensor_scalar(out=mean[:], in0=stt[:, 0:1],
                                        scalar1=inv_n, scalar2=None,
                                        op0=mybir.AluOpType.mult)
                var = work.tile([P, 1], dt.float32, tag="var")
                nc.vector.tensor_scalar(out=var[:], in0=stt[:, 1:2],
                                        scalar1=inv_n, scalar2=None,
                                        op0=mybir.AluOpType.mult)
                msq = work.tile([P, 1], dt.float32, tag="msq")
                nc.vector.tensor_tensor(out=msq[:], in0=mean[:], in1=mean[:],
                                        op=mybir.AluOpType.mult)
                nc.vector.tensor_tensor(out=var[:], in0=var[:], in1=msq[:],
                                        op=mybir.AluOpType.subtract)
                nc.vector.tensor_scalar(out=var[:], in0=var[:],
                                        scalar1=BN_EPS, scalar2=None,
                                        op0=mybir.AluOpType.add)
                sd = work.tile([P, 1], dt.float32, tag="sd")
                nc.scalar.activation(sd[:], var[:],
                                     mybir.ActivationFunctionType.Sqrt)
                inv = work.tile([P, 1], dt.float32, tag="inv")
                nc.vector.reciprocal(inv[:], sd[:])
                a_t = work.tile([P, 1], dt.float32, tag="a")
                nc.vector.tensor_tensor(out=a_t[:], in0=inv[:], in1=gac,
                                        op=mybir.AluOpType.mult)
                c_t = work.tile([P, 1], dt.float32, tag="c")
                nc.vector.tensor_tensor(out=c_t[:], in0=mean[:], in1=a_t[:],
                                        op=mybir.AluOpType.mult)
                nc.vector.tensor_tensor(out=c_t[:], in0=bec, in1=c_t[:],
                                        op=mybir.AluOpType.subtract)

                # ---- phase C: normalize, transpose, pool (HW loop) ----
                pool_sb = statp.tile([P, P], dt.float32, tag=f"pool{l}{rep}")
                nc.vector.memset(pool_sb[:], 0.0)
                with tc.For_i(0, T, 1) as t:
                    # walrus matmul lhsT needs a static offset: stage the
                    # normalized tile in a fixed SBUF buffer for the transpose
                    znf = work.tile([P, P], dt.float32, tag="znf")
                    if l < L - 1:
                        nc.scalar.activation(
                            h_loc[:, ds(t * P, P)], h_loc[:, ds(t * P, P)],
                            mybir.ActivationFunctionType.Identity,
                            bias=c_t[:, 0:1], scale=a_t[:, 0:1])
                        nc.scalar.copy(znf[:], h_loc[:, ds(t * P, P)])
                    else:
                        nc.scalar.activation(
                            znf[:], h_loc[:, ds(t * P, P)],
                            mybir.ActivationFunctionType.Identity,
                            bias=c_t[:, 0:1], scale=a_t[:, 0:1])
                    zT_ps = psT.tile([P, P], dt.float32, tag="zT")
                    nc.tensor.transpose(zT_ps[:], znf[:], ident_t[:])
                    znm = work.tile([P, P], dt.float32, tag="znm")
                    nc.scalar.copy(znm[:], zT_ps[:])
                    if l < L - 1:
                        nc.sync.dma_start(zz[l].ap()[ds(t * P, P), :], znm[:])
                    sb_t = spool.tile([P, P], dt.float32, tag="SB")
                    nc.vector.tensor_scalar(
                        out=sb_t[:], in0=iota_t[:],
                        scalar1=brel_t[:, ds(t, 1)], scalar2=None,
                        op0=mybir.AluOpType.is_equal)
                    pp = psP.tile([P, P], dt.float32, tag="pp")
                    nc.tensor.matmul(pp[:], lhsT=sb_t[:], rhs=znm[:],
                                     start=True, stop=True)
                    nc.vector.tensor_add(pool_sb[:], pool_sb[:], pp[:])
                nc.sync.dma_start(pools_d.ap()[:, l * F:(l + 1) * F],
                                  pool_sb[:])

                if l < L - 1 and not no_cc:
                    nc.gpsimd.collective_compute(
                        "AllGather", mybir.AluOpType.bypass,
                        replica_groups=groups,
                        ins=[zz[l].ap().opt()], outs=[hf[l].ap().opt()])

    nc.compile()
    return nc


def build_kernel(cfg, BCPT, **kw):
    return build_kernel_v2(cfg, BCPT, **kw)


# ---------------- v1 (fully unrolled; kept for reference/fallback) ----------
def build_kernel_v1(cfg, BCPT, num_swdge_queues=4, repeat=1, loop_n=1, no_cc=False, no_gather=False):
    CAPB = BCPT * 128
    T = cfg.T
    dt = mybir.dt
    nc = bacc.Bacc("TRN2", target_bir_lowering=False, debug=False,
                   enable_asserts=True, num_devices=NCORES,
                   num_swdge_queues=num_swdge_queues)

    # inputs
    xT_d = nc.dram_tensor("xT", [P, cfg.COLS], dt.float32, kind="ExternalInput")
    xfull_d = nc.dram_tensor("xfull", [cfg.HF_ROWS, F], dt.float32,
                             kind="ExternalInput")
    gidx_d = nc.dram_tensor("gidx", [P, T * 4 * (CAPB // 16)], dt.int16,
                            kind="ExternalInput")
    dstrel_d = nc.dram_tensor("dstrel", [P, T * 4 * BCPT], dt.float32,
                              kind="ExternalInput")
    brel_d = nc.dram_tensor("brel", [P, T], dt.float32, kind="ExternalInput")
    filled_d = nc.dram_tensor("filled", [P, T], dt.float32, kind="ExternalInput")
    w1_d = nc.dram_tensor("w1", [P, L * F], dt.float32, kind="ExternalInput")
    w2_d = nc.dram_tensor("w2", [P, L * F], dt.float32, kind="ExternalInput")
    bias_d = nc.dram_tensor("bias", [P, 4 * L], dt.float32, kind="ExternalInput")
    iota_d = nc.dram_tensor("iotat", [P, P], dt.float32, kind="ExternalInput")
    ident_d = nc.dram_tensor("ident", [P, P], dt.float32, kind="ExternalInput")
    pools_d = nc.dram_tensor("pools", [P, L * F], dt.float32,
                             kind="ExternalOutput")

    # internal DRAM
    zz = [nc.dram_tensor(f"zz{l}", [cfg.COLS, F], dt.float32, kind="Internal")
          for l in range(L - 1)]
    hf = [nc.dram_tensor(f"hf{l}", [cfg.HF_ROWS, F], dt.float32,
                         kind="Internal", addr_space="Shared")
          for l in range(L - 1)]
    st_in = [nc.dram_tensor(f"stin{l}", [P, 2], dt.float32, kind="Internal")
             for l in range(L)]
    st_out = [nc.dram_tensor(f"stout{l}", [P, 2], dt.float32, kind="Internal",
                             addr_space="Shared") for l in range(L)]

    inv_n = 1.0 / cfg.N
    groups = [list(range(NCORES))]

    with tile.TileContext(nc) as tc:
        with tc.tile_pool(name="big", bufs=1) as big, \
             tc.tile_pool(name="gpool", bufs=3) as gpool, \
             tc.tile_pool(name="spool", bufs=8) as spool, \
             tc.tile_pool(name="work", bufs=4) as work, \
             tc.tile_pool(name="stat", bufs=1) as statp, \
             tc.tile_pool(name="psA", bufs=2, space="PSUM") as psA, \
             tc.tile_pool(name="psT", bufs=2, space="PSUM") as psT, \
             tc.tile_pool(name="psM", bufs=2, space="PSUM") as psM, \
             tc.tile_pool(name="psP", bufs=1, space="PSUM") as psP:

            nc.gpsimd.load_library(library_config.mlp)

            h_loc = big.tile([P, cfg.COLS], dt.float32)       # feature-major h
            nc.sync.dma_start(h_loc[:], xT_d.ap())
            gidx_t = big.tile([P, T * 4 * (CAPB // 16)], dt.int16)
            nc.sync.dma_start(gidx_t[:], gidx_d.ap())
            drel_t = big.tile([P, T * 4 * BCPT], dt.float32)
            nc.sync.dma_start(drel_t[:], dstrel_d.ap())
            brel_t = big.tile([P, T], dt.float32)
            nc.sync.dma_start(brel_t[:], brel_d.ap())
            filled_t = big.tile([P, T], dt.float32)
            nc.sync.dma_start(filled_t[:], filled_d.ap())
            w1_t = big.tile([P, L * F], dt.float32)
            nc.sync.dma_start(w1_t[:], w1_d.ap())
            w2_t = big.tile([P, L * F], dt.float32)
            nc.sync.dma_start(w2_t[:], w2_d.ap())
            bias_t = big.tile([P, 4 * L], dt.float32)
            nc.sync.dma_start(bias_t[:], bias_d.ap())
            iota_t = big.tile([P, P], dt.float32)
            nc.sync.dma_start(iota_t[:], iota_d.ap())
            ident_t = big.tile([P, P], dt.float32)
            nc.sync.dma_start(ident_t[:], ident_d.ap())

            from contextlib import nullcontext
            with (tc.For_i(0, loop_n, 1) if loop_n > 1 else nullcontext()):
             for rep in range(repeat):
              for l in range(L):
                hsrc = xfull_d if (l == 0 or no_cc) else hf[l - 1]
                b1c = bias_t[:, 0 * L + l:0 * L + l + 1]
                b2c = bias_t[:, 1 * L + l:1 * L + l + 1]
                gac = bias_t[:, 2 * L + l:2 * L + l + 1]
                bec = bias_t[:, 3 * L + l:3 * L + l + 1]
                w1c = w1_t[:, l * F:(l + 1) * F]
                w2c = w2_t[:, l * F:(l + 1) * F]

                ssum = statp.tile([P, T], dt.float32, tag=f"ssum{l}")
                ssq = statp.tile([P, T], dt.float32, tag=f"ssq{l}")

                for t in range(T):
                    g_t = gpool.tile([P, 4 * BCPT, P], dt.float32, tag="G")
                    if no_gather:
                        nc.scalar.copy(g_t[:, 0, :], iota_t[:])
                    for b in range(4 if not no_gather else 0):
                        call = t * 4 + b
                        nc.gpsimd.dma_gather(
                            out_ap=g_t[:, b * BCPT:(b + 1) * BCPT, :],
                            in_ap=hsrc.ap()[b * cfg.BANK_ROWS:
                                            (b + 1) * cfg.BANK_ROWS, :],
                            idxs_ap=gidx_t[:, call * (CAPB // 16):
                                           (call + 1) * (CAPB // 16)],
                            num_idxs=CAPB,
                            num_idxs_reg=CAPB,
                            elem_size=F,
                            queue_num=b % num_swdge_queues,
                        )
                    agg_ps = psA.tile([P, P], dt.float32, tag="agg")
                    nch = 4 * BCPT
                    for ch in range(nch):
                        s_t = spool.tile([P, P], dt.float32, tag="S")
                        nc.vector.tensor_scalar(
                            out=s_t[:], in0=iota_t[:],
                            scalar1=drel_t[:, t * nch + ch:t * nch + ch + 1],
                            scalar2=None, op0=mybir.AluOpType.is_equal)
                        nc.tensor.matmul(agg_ps[:], lhsT=s_t[:],
                                         rhs=g_t[:, 0 if no_gather else ch, :],
                                         start=(ch == 0), stop=(ch == nch - 1))
                    # node-major agg -> SBUF -> transpose to feature-major
                    agg_nm = work.tile([P, P], dt.float32, tag="aggnm")
                    nc.scalar.copy(agg_nm[:], agg_ps[:])
                    aggT_ps = psT.tile([P, P], dt.float32, tag="aggT")
                    nc.tensor.transpose(aggT_ps[:], agg_nm[:], ident_t[:])
                    z1in = work.tile([P, P], dt.float32, tag="z1in")
                    nc.vector.tensor_add(z1in[:], h_loc[:, t * P:(t + 1) * P],
                                         aggT_ps[:])
                    # MLP (feature-major, weights stationary)
                    mp1 = psM.tile([P, P], dt.float32, tag="mp")
                    nc.tensor.matmul(mp1[:], lhsT=w1c, rhs=z1in[:],
                                     start=True, stop=True)
                    z1 = work.tile([P, P], dt.float32, tag="z1")
                    nc.scalar.activation(z1[:], mp1[:],
                                         mybir.ActivationFunctionType.Relu,
                                         bias=b1c)
                    mp2 = psM.tile([P, P], dt.float32, tag="mp")
                    nc.tensor.matmul(mp2[:], lhsT=w2c, rhs=z1[:],
                                     start=True, stop=True)
                    # z_pre overwrites h_loc tile in place
                    nc.scalar.activation(h_loc[:, t * P:(t + 1) * P], mp2[:],
                                         mybir.ActivationFunctionType.Relu,
                                         bias=b2c)
                    # zero phantom (unfilled + pad) seat columns, then stats
                    msk = spool.tile([P, P], dt.float32, tag="S")
                    nc.vector.tensor_scalar(
                        out=msk[:], in0=iota_t[:],
                        scalar1=filled_t[:, t:t + 1], scalar2=None,
                        op0=mybir.AluOpType.is_lt)
                    nc.vector.tensor_tensor(
                        out=h_loc[:, t * P:(t + 1) * P],
                        in0=h_loc[:, t * P:(t + 1) * P], in1=msk[:],
                        op=mybir.AluOpType.mult)
                    seat_ap = h_loc[:, t * P:(t + 1) * P]
                    nc.vector.tensor_reduce(
                        out=ssum[:, t:t + 1], in_=seat_ap,
                        axis=mybir.AxisListType.X, op=mybir.AluOpType.add)
                    sqs = work.tile([P, P], dt.float32, tag="sqs")
                    nc.scalar.activation(sqs[:], seat_ap,
                                         mybir.ActivationFunctionType.Square,
                                         accum_out=ssq[:, t:t + 1])

                # ---- BN stats allreduce ----
                red = work.tile([P, 2], dt.float32, tag="red")
                nc.vector.tensor_reduce(out=red[:, 0:1], in_=ssum[:],
                                        axis=mybir.AxisListType.X,
                                        op=mybir.AluOpType.add)
                nc.vector.tensor_reduce(out=red[:, 1:2], in_=ssq[:],
                                        axis=mybir.AxisListType.X,
                                        op=mybir.AluOpType.add)
                nc.sync.dma_start(st_in[l].ap(), red[:])
                if not no_cc:
                    nc.gpsimd.collective_compute(
                        "AllReduce", mybir.AluOpType.add, replica_groups=groups,
                        ins=[st_in[l].ap().opt()], outs=[st_out[l].ap().opt()])
                stt = work.tile([P, 2], dt.float32, tag="stt")
                nc.sync.dma_start(stt[:], (st_in[l] if no_cc else st_out[l]).ap())
                # mean, var, scale a, shift c
                mean = work.tile([P, 1], dt.float32, tag="mean")
                nc.vector.tensor_scalar(out=mean[:], in0=stt[:, 0:1],
                                        scalar1=inv_n, scalar2=None,
                                        op0=mybir.AluOpType.mult)
                var = work.tile([P, 1], dt.float32, tag="var")
                nc.vector.tensor_scalar(out=var[:], in0=stt[:, 1:2],
                                        scalar1=inv_n, scalar2=None,
                                        op0=mybir.AluOpType.mult)
                msq = work.tile([P, 1], dt.float32, tag="msq")
                nc.vector.tensor_tensor(out=msq[:], in0=mean[:], in1=mean[:],
                                        op=mybir.AluOpType.mult)
                nc.vector.tensor_tensor(out=var[:], in0=var[:], in1=msq[:],
                                        op=mybir.AluOpType.subtract)
                nc.vector.tensor_scalar(out=var[:], in0=var[:],
                                        scalar1=BN_EPS, scalar2=None,
                                        op0=mybir.AluOpType.add)
                sd = work.tile([P, 1], dt.float32, tag="sd")
                nc.scalar.activation(sd[:], var[:],
                                     mybir.ActivationFunctionType.Sqrt)
                inv = work.tile([P, 1], dt.float32, tag="inv")
                nc.vector.reciprocal(inv[:], sd[:])
                a_t = work.tile([P, 1], dt.float32, tag="a")
                nc.vector.tensor_tensor(out=a_t[:], in0=inv[:], in1=gac,
                                        op=mybir.AluOpType.mult)
                c_t = work.tile([P, 1], dt.float32, tag="c")
                nc.vector.tensor_tensor(out=c_t[:], in0=mean[:], in1=a_t[:],
                                        op=mybir.AluOpType.mult)
                nc.vector.tensor_tensor(out=c_t[:], in0=bec, in1=c_t[:],
                                        op=mybir.AluOpType.subtract)

                # ---- normalize in place, transpose, pool, store ----
                pool_ps = psP.tile([P, P], dt.float32, tag="pool")
                for t in range(T):
                    nc.scalar.activation(
                        h_loc[:, t * P:(t + 1) * P],
                        h_loc[:, t * P:(t + 1) * P],
                        mybir.ActivationFunctionType.Identity,
                        bias=c_t[:, 0:1], scale=a_t[:, 0:1])
                    zT_ps = psT.tile([P, P], dt.float32, tag="aggT")
                    nc.tensor.transpose(zT_ps[:], h_loc[:, t * P:(t + 1) * P],
                                        ident_t[:])
                    znm = work.tile([P, P], dt.float32, tag="znm")
                    nc.scalar.copy(znm[:], zT_ps[:])
                    if l < L - 1:
                        nc.sync.dma_start(zz[l].ap()[t * P:(t + 1) * P, :],
                                          znm[:])
                    sb_t = spool.tile([P, P], dt.float32, tag="S")
                    nc.vector.tensor_scalar(
                        out=sb_t[:], in0=iota_t[:],
                        scalar1=brel_t[:, t:t + 1], scalar2=None,
                        op0=mybir.AluOpType.is_equal)
                    nc.tensor.matmul(pool_ps[:], lhsT=sb_t[:], rhs=znm[:],
                                     start=(t == 0), stop=(t == T - 1),
                                     skip_group_check=True)
                poolsb = work.tile([P, P], dt.float32, tag="poolsb")
                nc.scalar.copy(poolsb[:], pool_ps[:])
                nc.sync.dma_start(pools_d.ap()[:, l * F:(l + 1) * F],
                                  poolsb[:])

                if l < L - 1 and not no_cc:
                    nc.gpsimd.collective_compute(
                        "AllGather", mybir.AluOpType.bypass,
                        replica_groups=groups,
                        ins=[zz[l].ap().opt()], outs=[hf[l].ap().opt()])

    nc.compile()
    return nc


def make_in_maps(per_core, x_full, inputs):
    W1 = np.asarray(inputs['W1'], np.float32)
    W2 = np.asarray(inputs['W2'], np.float32)
    b1 = np.asarray(inputs['b1'], np.float32)
    b2 = np.asarray(inputs['b2'], np.float32)
    gamma = np.asarray(inputs['gamma'], np.float32)
    beta = np.asarray(inputs['beta'], np.float32)
    w1_h = np.ascontiguousarray(np.concatenate([W1[i] for i in range(L)], 1))
    w2_h = np.ascontiguousarray(np.concatenate([W2[i] for i in range(L)], 1))
    bias_h = np.ascontiguousarray(
        np.concatenate([b1.T, b2.T, gamma.T, beta.T], 1))
    iota_h = np.tile(np.arange(P, dtype=np.float32), (P, 1))
    ident_h = np.eye(P, dtype=np.float32)
    in_maps = []
    for c in range(NCORES):
        pc = per_core[c]
        in_maps.append({
            "xT": pc["xT"], "gidx": pc["gidx"],
            "dstrel": pc["dstrel"], "brel": pc["batch_rel"],
            "filled": pc["filled"],
            "w1": w1_h, "w2": w2_h, "bias": bias_h,
            "iotat": iota_h, "ident": ident_h,
        })
    return in_maps


def build_null_kernel(cfg, BCPT):
    """Same I/O signature as the real kernel, trivial body. Used to measure
    the non-compute overhead (upload/dispatch) of a run for timing deltas."""
    CAPB = BCPT * 128
    T = cfg.T
    dt = mybir.dt
    nc = bacc.Bacc("TRN2", target_bir_lowering=False, debug=False,
                   enable_asserts=True, num_devices=NCORES,
                   num_swdge_queues=4)
    xT_d = nc.dram_tensor("xT", [P, cfg.COLS], dt.float32,
                          kind="ExternalInput")
    nc.dram_tensor("gidx", [P, T * 4 * (CAPB // 16)], dt.int16,
                   kind="ExternalInput")
    nc.dram_tensor("dstrel", [P, T * 4 * BCPT], dt.float32,
                   kind="ExternalInput")
    nc.dram_tensor("brel", [P, T], dt.float32, kind="ExternalInput")
    nc.dram_tensor("filled", [P, T], dt.float32, kind="ExternalInput")
    nc.dram_tensor("w1", [P, L * F], dt.float32, kind="ExternalInput")
    nc.dram_tensor("w2", [P, L * F], dt.float32, kind="ExternalInput")
    nc.dram_tensor("bias", [P, 4 * L], dt.float32, kind="ExternalInput")
    nc.dram_tensor("iotat", [P, P], dt.float32, kind="ExternalInput")
    nc.dram_tensor("ident", [P, P], dt.float32, kind="ExternalInput")
    pools_d = nc.dram_tensor("pools", [P, L * F], dt.float32,
                             kind="ExternalOutput")
    with tile.TileContext(nc) as tc:
        with tc.tile_pool(name="sb", bufs=1) as sb:
            t0 = sb.tile([P, L * F], dt.float32)
            nc.sync.dma_start(t0[:], xT_d.ap()[:, 0:L * F])
            nc.sync.dma_start(pools_d.ap(), t0[:])
    nc.compile()
    return nc


# ---------------- entry point ----------------
def kernel(x, edge_index, batch, W1, b1, W2, b2, gamma, beta):
    cfg = FULL
    x = np.asarray(x, np.float32)
    edge_index = np.asarray(edge_index, np.int32)
    batch = np.asarray(batch, np.int32)
    W1 = np.asarray(W1, np.float32)
    b1 = np.asarray(b1, np.float32)
    W2 = np.asarray(W2, np.float32)
    b2 = np.asarray(b2, np.float32)
    gamma = np.asarray(gamma, np.float32)
    beta = np.asarray(beta, np.float32)

    x_full, per_core, meta = preprocess(cfg, x, edge_index, batch)
    nc = build_kernel(cfg, meta['BCPT'])
    in_maps = make_in_maps(per_core, x_full, dict(
        W1=W1, W2=W2, b1=b1, b2=b2, gamma=gamma, beta=beta))

    import time as _time
    last_exc = None
    for attempt in range(3):
        try:
            res = bass_utils.run_bass_kernel_spmd(
                nc, in_maps, core_ids=list(range(NCORES)))
            break
        except Exception as e:       # transient device wedge -> retry
            last_exc = e
            _time.sleep(20)
    else:
        raise last_exc

    out = np.zeros((NUM_GRAPHS, L * F), np.float32)
    for c in range(NCORES):
        pool_c = res.results[c]["pools"]          # [128, L*F]
        g0, sp = meta['g_bases'][c], meta['spans'][c]
        out[g0:g0 + sp] += pool_c[:sp]
    return out


if __name__ == "__main__":
    import reference
    inputs = reference.setup_inputs()
    inputs = {k: np.asarray(v) for k, v in inputs.items()}
    got = kernel(**inputs)
    print("kernel output shape:", got.shape)



# revision 21
# speedup vs baseline: 37.4780x; 1.0714x over previous
"""GIN encoder (3x GINConv+BN + per-layer global_add_pool) on 8 Trainium2 cores.

Strategy:
  - Nodes sharded round 8 cores (12500 each). Each core's nodes are permuted
    into 102 "tiles" of 125 seats (+3 pad) chosen to balance gather traffic
    across 4 source banks (degree-sorted round-robin + local repair).
  - Edges sharded by dst owner; per (dst-tile, src-bank) edge slots padded to
    a fixed cap -> fully uniform SPMD program.
  - Aggregation h[src] gather via dma_gather (4 SWDGE queues) from a
    [104448, 128] node-feature table (AllGather of per-core node-major
    slices for layer 0, AllGather outputs for layers 1, 2), then
    segment-sum as one-hot matmuls accumulating feature-major in PSUM.
  - MLP feature-major on PE (weights stationary), BN stats via [128,2]
    AllReduce, pool via one-hot matmul; host sums per-core partial pools.
  - All per-tile pipelines run inside For_i hardware loops with
    register-indexed (DynSlice) APs: ~1.1k static instructions vs ~24k
    fully unrolled. This runtime charges ~130us per static instruction
    per execution, so static code size dominates device time; hardware
    loop re-execution is nearly free (measured ~10ns/instruction).
"""
import sys
sys.path.insert(0, '/opt/trn_rl_repo')

import numpy as np

import concourse.bass as bass
import concourse.tile as tile
from concourse import bacc, mybir, library_config
from concourse import bass_utils

# ---------------- problem config (hardcoded from spec) ----------------
NCORES = 8
N = 100000
F = 128
E = 1600000
L = 3
NUM_GRAPHS = 512
BN_EPS = 1e-5
P = 128


class Cfg:
    def __init__(self, N, E, num_graphs, T, seats, capb):
        self.N = N
        self.E = E
        self.num_graphs = num_graphs
        self.NPC = N // NCORES            # nodes per core
        self.T = T                        # dst tiles per core
        self.SEATS = seats                # real seats per tile (<=128)
        self.CAPB = capb                  # slots per (tile, bank); %128==0
        self.BCPT = capb // 128           # chunks per (tile, bank)
        self.COLS = T * P                 # padded node columns per core
        self.HF_ROWS = NCORES * self.COLS
        self.NBANK = 4
        self.BANK_ROWS = self.HF_ROWS // 4
        assert self.BANK_ROWS < 32767
        assert self.T * self.SEATS >= self.NPC
        assert self.CAPB <= 1024 and self.CAPB % 128 == 0


FULL = Cfg(N, E, NUM_GRAPHS, T=102, seats=125, capb=512)


# ---------------- host-side preprocessing ----------------
def preprocess(cfg, x, edge_index, batch):
    """Returns per-core input dicts + metadata for output assembly."""
    NPC, T, SEATS = cfg.NPC, cfg.T, cfg.SEATS
    src = edge_index[0].astype(np.int64)
    dst = edge_index[1].astype(np.int64)
    owner = dst // NPC
    src_bank = src // (2 * NPC)          # = owner(src)//2, permutation-invariant

    # --- per-core tile assignment balancing per-bank degree ---
    tile_of_g = np.empty(cfg.N, np.int64)     # local tile of each node
    seat_of_g = np.empty(cfg.N, np.int64)     # seat within tile
    capb_used = cfg.CAPB
    for c in range(NCORES):
        m = owner == c
        d_loc = dst[m] - c * NPC
        counts = np.zeros((NPC, 4), np.int64)
        np.add.at(counts, (d_loc, src_bank[m]), 1)
        deg = counts.sum(1)
        order = np.argsort(-deg, kind='stable')

        # fast path: degree-sorted round-robin + local repair of cap
        # violations; falls back to the exact greedy below if repair fails
        ok_all = False
        tile_of = np.empty(NPC, np.int64)
        tile_of[order] = np.arange(NPC) % T
        loads = np.zeros((T, 4), np.int64)
        for b in range(4):
            loads[:, b] = np.bincount(tile_of, weights=counts[:, b],
                                      minlength=T).astype(np.int64)
        nseat = np.bincount(tile_of, minlength=T)
        if nseat.max() <= SEATS:
            for _repair in range(200):
                over = np.argwhere(loads > cfg.CAPB)
                if len(over) == 0:
                    ok_all = True
                    break
                t_bad, b_bad = over[0]
                cand = np.where(tile_of == t_bad)[0]
                cand = cand[np.argsort(-counts[cand, b_bad])]
                moved = False
                for d in cand:
                    room = ((loads + counts[d] <= cfg.CAPB).all(1)
                            & (nseat < SEATS))
                    room[t_bad] = False
                    tgt = np.argwhere(room)
                    if len(tgt):
                        t_new = int(tgt[0][0])
                        loads[t_bad] -= counts[d]
                        loads[t_new] += counts[d]
                        nseat[t_bad] -= 1
                        nseat[t_new] += 1
                        tile_of[d] = t_new
                        moved = True
                        break
                if not moved:
                    break
        if ok_all:
            seat_of = np.empty(NPC, np.int64)
            order_t = np.argsort(tile_of, kind='stable')
            tt = tile_of[order_t]
            starts = np.searchsorted(tt, np.arange(T))
            seat_of[order_t] = np.arange(NPC) - np.repeat(
                starts, np.bincount(tt, minlength=T))
            tile_of_g[c * NPC:(c + 1) * NPC] = tile_of
            seat_of_g[c * NPC:(c + 1) * NPC] = seat_of
            continue

        for cap_try in (cfg.CAPB, cfg.CAPB + 128, cfg.CAPB + 256, 1024):
            rem = np.full((T, 4), cap_try, np.int64)
            seats = np.full(T, SEATS, np.int64)
            tile_of = np.full(NPC, -1, np.int64)
            seat_of = np.full(NPC, -1, np.int64)
            ok_all = True
            for d in order:
                v = counts[d]
                feas = (seats > 0) & (rem >= v).all(1)
                if not feas.any():
                    ok_all = False
                    break
                slack = (rem - v).min(1).astype(np.float64)
                slack[~feas] = -1e18
                t = int(np.argmax(slack))
                tile_of[d] = t
                seat_of[d] = SEATS - seats[t]
                rem[t] -= v
                seats[t] -= 1
            if ok_all:
                capb_used = max(capb_used, cap_try)
                break
        assert ok_all, "tile assignment failed even at cap 1024"
        tile_of_g[c * NPC:(c + 1) * NPC] = tile_of
        seat_of_g[c * NPC:(c + 1) * NPC] = seat_of

    CAPB = capb_used
    BCPT = CAPB // 128
    perm_pos = tile_of_g * P + seat_of_g                     # within-core col
    perm_row = (np.arange(cfg.N) // NPC) * cfg.COLS + perm_pos  # global row

    # --- x tables ---
    xf = np.asarray(x, np.float32)
    x_full = np.zeros((cfg.HF_ROWS, F), np.float32)
    x_full[perm_row] = xf
    g_bases, spans = [], []
    per_core = []
    for c in range(NCORES):
        lo, hi = c * NPC, (c + 1) * NPC
        xnm = np.zeros((cfg.COLS, F), np.float32)            # node-major
        xnm[perm_pos[lo:hi]] = xf[lo:hi]
        xlT = np.ascontiguousarray(xnm.T)                    # [128, COLS]

        g_base = int(batch[lo])
        span = int(batch[hi - 1]) - g_base + 1
        assert span <= P, f"graph span {span} exceeds 128"
        g_bases.append(g_base)
        spans.append(span)
        br = np.full(cfg.COLS, -1.0, np.float32)
        br[perm_pos[lo:hi]] = (batch[lo:hi] - g_base).astype(np.float32)
        batch_rel = np.ascontiguousarray(br.reshape(T, P).T)  # [128, T]
        filled_row = np.bincount(tile_of_g[lo:hi], minlength=T).astype(np.float32)
        filled_tbl = np.tile(filled_row, (P, 1))              # [128, T]

        # --- edge slot tables ---
        m = owner == c
        e_src = src[m]
        e_dst = dst[m] - c * NPC
        key = tile_of_g[c * NPC + e_dst] * 4 + src_bank[m]
        order_e = np.argsort(key, kind='stable')
        key_s = key[order_e]
        cnt = np.bincount(key_s, minlength=T * 4)
        assert cnt.max() <= CAPB, f"(tile,bank) count {cnt.max()} > cap {CAPB}"
        cstart = np.zeros(T * 4, np.int64)
        cstart[1:] = np.cumsum(cnt)[:-1]
        within = np.arange(len(key_s)) - np.repeat(cstart, cnt)
        slot = key_s * CAPB + within
        gidx_flat = np.zeros(T * 4 * CAPB, np.int64)
        drel_flat = np.full(T * 4 * CAPB, -1.0, np.float32)
        gidx_flat[slot] = perm_row[e_src[order_e]] % cfg.BANK_ROWS
        drel_flat[slot] = seat_of_g[c * NPC + e_dst[order_e]]
        # idx wrap: per call (t,b): CAPB values -> [16, CAPB//16], tile x8
        w = gidx_flat.reshape(T * 4, CAPB // 16, 16).transpose(2, 0, 1)
        w = w.reshape(16, T * 4 * (CAPB // 16))
        gidx_h = np.tile(w, (8, 1)).astype(np.int16)         # [128, cols]
        drel_h = np.ascontiguousarray(
            drel_flat.reshape(T * 4 * BCPT, P).T)             # [128, chunks]

        per_core.append(dict(xT=xlT, xnm=xnm, batch_rel=batch_rel,
                             gidx=gidx_h, dstrel=drel_h, filled=filled_tbl))

    meta = dict(CAPB=CAPB, BCPT=BCPT, g_bases=g_bases, spans=spans)
    return x_full, per_core, meta


# ---------------- device kernel builder (v2: hardware loops) ----------------
def build_kernel_v2(cfg, BCPT, num_swdge_queues=4, repeat=1, no_cc=False,
                    fat_onehot=True, t_frac=1.0):
    """Same algorithm as v1 but the per-tile pipelines run inside For_i
    hardware loops with register-indexed APs. Static instruction count
    ~220 vs ~15.6k; this runtime charges ~127us per STATIC instruction,
    so this is the dominant win."""
    from concourse.bass import ds
    CAPB = BCPT * 128
    T = cfg.T
    nch = 4 * BCPT
    C16 = CAPB // 16
    dt = mybir.dt
    nc = bacc.Bacc("TRN2", target_bir_lowering=False, debug=False,
                   enable_asserts=True, num_devices=NCORES,
                   num_swdge_queues=num_swdge_queues)

    # inputs (v1 signature minus the replicated xfull table: the layer-0
    # gather table is AllGathered on device from the per-core xT slices)
    xT_d = nc.dram_tensor("xT", [P, cfg.COLS], dt.float32, kind="ExternalInput")
    gidx_d = nc.dram_tensor("gidx", [P, T * 4 * C16], dt.int16,
                            kind="ExternalInput")
    dstrel_d = nc.dram_tensor("dstrel", [P, T * nch], dt.float32,
                              kind="ExternalInput")
    xnm_in = nc.dram_tensor("xnm", [cfg.COLS, F], dt.float32,
                            kind="ExternalInput")
    xnm_d = nc.dram_tensor("xnmi", [cfg.COLS, F], dt.float32, kind="Internal")
    xg_d = nc.dram_tensor("xg", [cfg.HF_ROWS, F], dt.float32,
                          kind="Internal", addr_space="Shared")
    brel_d = nc.dram_tensor("brel", [P, T], dt.float32, kind="ExternalInput")
    filled_d = nc.dram_tensor("filled", [P, T], dt.float32, kind="ExternalInput")
    w1_d = nc.dram_tensor("w1", [P, L * F], dt.float32, kind="ExternalInput")
    w2_d = nc.dram_tensor("w2", [P, L * F], dt.float32, kind="ExternalInput")
    bias_d = nc.dram_tensor("bias", [P, 4 * L], dt.float32, kind="ExternalInput")
    iota_d = nc.dram_tensor("iotat", [P, P], dt.float32, kind="ExternalInput")
    ident_d = nc.dram_tensor("ident", [P, P], dt.float32, kind="ExternalInput")
    pools_d = nc.dram_tensor("pools", [P, L * F], dt.float32,
                             kind="ExternalOutput")

    zz = [nc.dram_tensor(f"zz{l}", [cfg.COLS, F], dt.float32, kind="Internal")
          for l in range(L - 1)]
    hf = [nc.dram_tensor(f"hf{l}", [cfg.HF_ROWS, F], dt.float32,
                         kind="Internal", addr_space="Shared")
          for l in range(L - 1)]
    st_in = [nc.dram_tensor(f"stin{l}", [P, 2], dt.float32, kind="Internal")
             for l in range(L)]
    st_out = [nc.dram_tensor(f"stout{l}", [P, 2], dt.float32, kind="Internal",
                             addr_space="Shared") for l in range(L)]

    inv_n = 1.0 / cfg.N
    groups = [list(range(NCORES))]

    with tile.TileContext(nc) as tc:
        with tc.tile_pool(name="big", bufs=1) as big, \
             tc.tile_pool(name="gpool", bufs=2) as gpool, \
             tc.tile_pool(name="spool", bufs=2) as spool, \
             tc.tile_pool(name="work", bufs=2) as work, \
             tc.tile_pool(name="stat", bufs=1) as statp, \
             tc.tile_pool(name="psA", bufs=2, space="PSUM") as psA, \
             tc.tile_pool(name="psT", bufs=2, space="PSUM") as psT, \
             tc.tile_pool(name="psM", bufs=2, space="PSUM") as psM, \
             tc.tile_pool(name="psP", bufs=2, space="PSUM") as psP:

            nc.gpsimd.load_library(library_config.mlp)

            h_loc = big.tile([P, cfg.COLS], dt.float32)
            nc.sync.dma_start(h_loc[:], xT_d.ap())
            gidx_t = big.tile([P, T * 4 * C16], dt.int16)
            nc.sync.dma_start(gidx_t[:], gidx_d.ap())
            drel_t = big.tile([P, T * nch], dt.float32)
            nc.sync.dma_start(drel_t[:], dstrel_d.ap())
            brel_t = big.tile([P, T], dt.float32)
            nc.sync.dma_start(brel_t[:], brel_d.ap())
            filled_t = big.tile([P, T], dt.float32)
            nc.sync.dma_start(filled_t[:], filled_d.ap())
            w1_t = big.tile([P, L * F], dt.float32)
            nc.sync.dma_start(w1_t[:], w1_d.ap())
            w2_t = big.tile([P, L * F], dt.float32)
            nc.sync.dma_start(w2_t[:], w2_d.ap())
            bias_t = big.tile([P, 4 * L], dt.float32)
            nc.sync.dma_start(bias_t[:], bias_d.ap())
            iota_t = big.tile([P, P], dt.float32)
            nc.sync.dma_start(iota_t[:], iota_d.ap())
            ident_t = big.tile([P, P], dt.float32)
            nc.sync.dma_start(ident_t[:], ident_d.ap())

            # layer-0 gather table: AllGather the host-provided node-major x
            # (collective ins must be internal DRAM -> one local copy first)
            nc.sync.dma_start(xnm_d.ap(), xnm_in.ap())
            if not no_cc:
                nc.gpsimd.collective_compute(
                    "AllGather", mybir.AluOpType.bypass,
                    replica_groups=groups,
                    ins=[xnm_d.ap().opt()], outs=[xg_d.ap().opt()])

            for rep in range(repeat):
             for l in range(L):
                hsrc = xg_d if (l == 0 or no_cc) else hf[l - 1]
                b1c = bias_t[:, 0 * L + l:0 * L + l + 1]
                b2c = bias_t[:, 1 * L + l:1 * L + l + 1]
                gac = bias_t[:, 2 * L + l:2 * L + l + 1]
                bec = bias_t[:, 3 * L + l:3 * L + l + 1]
                w1c = w1_t[:, l * F:(l + 1) * F]
                w2c = w2_t[:, l * F:(l + 1) * F]

                acc = statp.tile([P, 2], dt.float32, tag=f"acc{l}{rep}")
                nc.vector.memset(acc[:], 0.0)

                # ---- phase A: aggregate + MLP, HW loop over dst tiles ----
                with tc.For_i(0, max(1, int(T * t_frac)), 1) as t:
                    g_t = gpool.tile([P, nch, P], dt.float32, tag="G")
                    for b in range(4):
                        nc.gpsimd.dma_gather(
                            out_ap=g_t[:, b * BCPT:(b + 1) * BCPT, :],
                            in_ap=hsrc.ap()[b * cfg.BANK_ROWS:
                                            (b + 1) * cfg.BANK_ROWS, :],
                            idxs_ap=gidx_t[:, ds(t * (4 * C16) + b * C16, C16)],
                            num_idxs=CAPB,
                            num_idxs_reg=CAPB,
                            elem_size=F,
                            queue_num=b % num_swdge_queues,
                        )
                    # one-hot seat matrices for all chunks
                    if fat_onehot:
                        s_all = spool.tile([P, nch, P], dt.float32, tag="S")
                        nc.vector.tensor_tensor(
                            out=s_all[:],
                            in0=iota_t[:].unsqueeze(1)
                                .broadcast_to([P, nch, P]),
                            in1=drel_t[:, ds(t * nch, nch)].unsqueeze(2)
                                .broadcast_to([P, nch, P]),
                            op=mybir.AluOpType.is_equal)
                    else:
                        s_all = spool.tile([P, nch, P], dt.float32, tag="S")
                        for ch in range(nch):
                            nc.vector.tensor_scalar(
                                out=s_all[:, ch, :], in0=iota_t[:],
                                scalar1=drel_t[:, ds(t * nch + ch, 1)],
                                scalar2=None, op0=mybir.AluOpType.is_equal)
                    # feature-major agg: agg[f, seat] += g[e,f]^T @ onehot[e,seat]
                    agg_ps = psA.tile([P, P], dt.float32, tag="agg")
                    for ch in range(nch):
                        nc.tensor.matmul(agg_ps[:], lhsT=g_t[:, ch, :],
                                         rhs=s_all[:, ch, :],
                                         start=(ch == 0), stop=(ch == nch - 1))
                    z1in = work.tile([P, P], dt.float32, tag="z1in")
                    nc.vector.tensor_add(z1in[:], h_loc[:, ds(t * P, P)],
                                         agg_ps[:])
                    mp1 = psM.tile([P, P], dt.float32, tag="mp")
                    nc.tensor.matmul(mp1[:], lhsT=w1c, rhs=z1in[:],
                                     start=True, stop=True)
                    z1 = work.tile([P, P], dt.float32, tag="z1")
                    nc.scalar.activation(z1[:], mp1[:],
                                         mybir.ActivationFunctionType.Relu,
                                         bias=b1c)
                    mp2 = psM.tile([P, P], dt.float32, tag="mp")
                    nc.tensor.matmul(mp2[:], lhsT=w2c, rhs=z1[:],
                                     start=True, stop=True)
                    nc.scalar.activation(h_loc[:, ds(t * P, P)], mp2[:],
                                         mybir.ActivationFunctionType.Relu,
                                         bias=b2c)
                    msk = work.tile([P, P], dt.float32, tag="msk")
                    nc.vector.tensor_scalar(
                        out=msk[:], in0=iota_t[:],
                        scalar1=filled_t[:, ds(t, 1)], scalar2=None,
                        op0=mybir.AluOpType.is_lt)
                    nc.vector.tensor_tensor(
                        out=h_loc[:, ds(t * P, P)],
                        in0=h_loc[:, ds(t * P, P)], in1=msk[:],
                        op=mybir.AluOpType.mult)
                    s1 = work.tile([P, 1], dt.float32, tag="s1")
                    nc.vector.tensor_reduce(
                        out=s1[:], in_=h_loc[:, ds(t * P, P)],
                        axis=mybir.AxisListType.X, op=mybir.AluOpType.add)
                    nc.vector.tensor_tensor(out=acc[:, 0:1], in0=acc[:, 0:1],
                                            in1=s1[:],
                                            op=mybir.AluOpType.add)
                    sqs = work.tile([P, P], dt.float32, tag="sqs")
                    s2 = work.tile([P, 1], dt.float32, tag="s2")
                    nc.scalar.activation(sqs[:], h_loc[:, ds(t * P, P)],
                                         mybir.ActivationFunctionType.Square,
                                         accum_out=s2[:])
                    nc.vector.tensor_tensor(out=acc[:, 1:2], in0=acc[:, 1:2],
                                            in1=s2[:],
                                            op=mybir.AluOpType.add)

                # ---- BN stats allreduce ----
                nc.sync.dma_start(st_in[l].ap(), acc[:])
                if not no_cc:
                    nc.gpsimd.collective_compute(
                        "AllReduce", mybir.AluOpType.add, replica_groups=groups,
                        ins=[st_in[l].ap().opt()], outs=[st_out[l].ap().opt()])
                stt = work.tile([P, 2], dt.float32, tag="stt")
                nc.sync.dma_start(stt[:], (st_in[l] if no_cc else st_out[l]).ap())
                mean = work.tile([P, 1], dt.float32, tag="mean")
                nc.vector.tensor_scalar(out=mean[:], in0=stt[:, 0:1],
                                        scalar1=inv_n, scalar2=None,
                                        op0=mybir.AluOpType.mult)
                var = work.tile([P, 1], dt.float32, tag="var")
                nc.vector.tensor_scalar(out=var[:], in0=stt[:, 1:2],
                                        scalar1=inv_n, scalar2=None,
                                        op0=mybir.AluOpType.mult)
                msq = work.tile([P, 1], dt.float32, tag="msq")
                nc.vector.tensor_tensor(out=msq[:], in0=mean[:], in1=mean[:],
                                        op=mybir.AluOpType.mult)
                nc.vector.tensor_tensor(out=var[:], in0=var[:], in1=msq[:],
                                        op=mybir.AluOpType.subtract)
                nc.vector.tensor_scalar(out=var[:], in0=var[:],
                                        scalar1=BN_EPS, scalar2=None,
                                        op0=mybir.AluOpType.add)
                sd = work.tile([P, 1], dt.float32, tag="sd")
                nc.scalar.activation(sd[:], var[:],
                                     mybir.ActivationFunctionType.Sqrt)
                inv = work.tile([P, 1], dt.float32, tag="inv")
                nc.vector.reciprocal(inv[:], sd[:])
                a_t = work.tile([P, 1], dt.float32, tag="a")
                nc.vector.tensor_tensor(out=a_t[:], in0=inv[:], in1=gac,
                                        op=mybir.AluOpType.mult)
                c_t = work.tile([P, 1], dt.float32, tag="c")
                nc.vector.tensor_tensor(out=c_t[:], in0=mean[:], in1=a_t[:],
                                        op=mybir.AluOpType.mult)
                nc.vector.tensor_tensor(out=c_t[:], in0=bec, in1=c_t[:],
                                        op=mybir.AluOpType.subtract)

                # ---- phase C: normalize, transpose, pool (HW loop) ----
                pool_sb = statp.tile([P, P], dt.float32, tag=f"pool{l}{rep}")
                nc.vector.memset(pool_sb[:], 0.0)
                with tc.For_i(0, max(1, int(T * t_frac)), 1) as t:
                    # walrus matmul lhsT needs a static offset: stage the
                    # normalized tile in a fixed SBUF buffer for the transpose
                    znf = work.tile([P, P], dt.float32, tag="znf")
                    if l < L - 1:
                        nc.scalar.activation(
                            h_loc[:, ds(t * P, P)], h_loc[:, ds(t * P, P)],
                            mybir.ActivationFunctionType.Identity,
                            bias=c_t[:, 0:1], scale=a_t[:, 0:1])
                        nc.scalar.copy(znf[:], h_loc[:, ds(t * P, P)])
                    else:
                        nc.scalar.activation(
                            znf[:], h_loc[:, ds(t * P, P)],
                            mybir.ActivationFunctionType.Identity,
                            bias=c_t[:, 0:1], scale=a_t[:, 0:1])
                    zT_ps = psT.tile([P, P], dt.float32, tag="zT")
                    nc.tensor.transpose(zT_ps[:], znf[:], ident_t[:])
                    znm = work.tile([P, P], dt.float32, tag="znm")
                    nc.scalar.copy(znm[:], zT_ps[:])
                    if l < L - 1:
                        nc.sync.dma_start(zz[l].ap()[ds(t * P, P), :], znm[:])
                    sb_t = spool.tile([P, P], dt.float32, tag="SB")
                    nc.vector.tensor_scalar(
                        out=sb_t[:], in0=iota_t[:],
                        scalar1=brel_t[:, ds(t, 1)], scalar2=None,
                        op0=mybir.AluOpType.is_equal)
                    pp = psP.tile([P, P], dt.float32, tag="pp")
                    nc.tensor.matmul(pp[:], lhsT=sb_t[:], rhs=znm[:],
                                     start=True, stop=True)
                    nc.vector.tensor_add(pool_sb[:], pool_sb[:], pp[:])
                nc.sync.dma_start(pools_d.ap()[:, l * F:(l + 1) * F],
                                  pool_sb[:])

                if l < L - 1 and not no_cc:
                    nc.gpsimd.collective_compute(
                        "AllGather", mybir.AluOpType.bypass,
                        replica_groups=groups,
                        ins=[zz[l].ap().opt()], outs=[hf[l].ap().opt()])

    nc.compile()
    return nc


def build_kernel(cfg, BCPT, **kw):
    return build_kernel_v2(cfg, BCPT, **kw)


# ---------------- v1 (fully unrolled; kept for reference/fallback) ----------
def build_kernel_v1(cfg, BCPT, num_swdge_queues=4, repeat=1, loop_n=1, no_cc=False, no_gather=False):
    CAPB = BCPT * 128
    T = cfg.T
    dt = mybir.dt
    nc = bacc.Bacc("TRN2", target_bir_lowering=False, debug=False,
                   enable_asserts=True, num_devices=NCORES,
                   num_swdge_queues=num_swdge_queues)

    # inputs
    xT_d = nc.dram_tensor("xT", [P, cfg.COLS], dt.float32, kind="ExternalInput")
    xfull_d = nc.dram_tensor("xfull", [cfg.HF_ROWS, F], dt.float32,
                             kind="ExternalInput")
    gidx_d = nc.dram_tensor("gidx", [P, T * 4 * (CAPB // 16)], dt.int16,
                            kind="ExternalInput")
    dstrel_d = nc.dram_tensor("dstrel", [P, T * 4 * BCPT], dt.float32,
                              kind="ExternalInput")
    brel_d = nc.dram_tensor("brel", [P, T], dt.float32, kind="ExternalInput")
    filled_d = nc.dram_tensor("filled", [P, T], dt.float32, kind="ExternalInput")
    w1_d = nc.dram_tensor("w1", [P, L * F], dt.float32, kind="ExternalInput")
    w2_d = nc.dram_tensor("w2", [P, L * F], dt.float32, kind="ExternalInput")
    bias_d = nc.dram_tensor("bias", [P, 4 * L], dt.float32, kind="ExternalInput")
    iota_d = nc.dram_tensor("iotat", [P, P], dt.float32, kind="ExternalInput")
    ident_d = nc.dram_tensor("ident", [P, P], dt.float32, kind="ExternalInput")
    pools_d = nc.dram_tensor("pools", [P, L * F], dt.float32,
                             kind="ExternalOutput")

    # internal DRAM
    zz = [nc.dram_tensor(f"zz{l}", [cfg.COLS, F], dt.float32, kind="Internal")
          for l in range(L - 1)]
    hf = [nc.dram_tensor(f"hf{l}", [cfg.HF_ROWS, F], dt.float32,
                         kind="Internal", addr_space="Shared")
          for l in range(L - 1)]
    st_in = [nc.dram_tensor(f"stin{l}", [P, 2], dt.float32, kind="Internal")
             for l in range(L)]
    st_out = [nc.dram_tensor(f"stout{l}", [P, 2], dt.float32, kind="Internal",
                             addr_space="Shared") for l in range(L)]

    inv_n = 1.0 / cfg.N
    groups = [list(range(NCORES))]

    with tile.TileContext(nc) as tc:
        with tc.tile_pool(name="big", bufs=1) as big, \
             tc.tile_pool(name="gpool", bufs=3) as gpool, \
             tc.tile_pool(name="spool", bufs=8) as spool, \
             tc.tile_pool(name="work", bufs=4) as work, \
             tc.tile_pool(name="stat", bufs=1) as statp, \
             tc.tile_pool(name="psA", bufs=2, space="PSUM") as psA, \
             tc.tile_pool(name="psT", bufs=2, space="PSUM") as psT, \
             tc.tile_pool(name="psM", bufs=2, space="PSUM") as psM, \
             tc.tile_pool(name="psP", bufs=1, space="PSUM") as psP:

            nc.gpsimd.load_library(library_config.mlp)

            h_loc = big.tile([P, cfg.COLS], dt.float32)       # feature-major h
            nc.sync.dma_start(h_loc[:], xT_d.ap())
            gidx_t = big.tile([P, T * 4 * (CAPB // 16)], dt.int16)
            nc.sync.dma_start(gidx_t[:], gidx_d.ap())
            drel_t = big.tile([P, T * 4 * BCPT], dt.float32)
            nc.sync.dma_start(drel_t[:], dstrel_d.ap())
            brel_t = big.tile([P, T], dt.float32)
            nc.sync.dma_start(brel_t[:], brel_d.ap())
            filled_t = big.tile([P, T], dt.float32)
            nc.sync.dma_start(filled_t[:], filled_d.ap())
            w1_t = big.tile([P, L * F], dt.float32)
            nc.sync.dma_start(w1_t[:], w1_d.ap())
            w2_t = big.tile([P, L * F], dt.float32)
            nc.sync.dma_start(w2_t[:], w2_d.ap())
            bias_t = big.tile([P, 4 * L], dt.float32)
            nc.sync.dma_start(bias_t[:], bias_d.ap())
            iota_t = big.tile([P, P], dt.float32)
            nc.sync.dma_start(iota_t[:], iota_d.ap())
            ident_t = big.tile([P, P], dt.float32)
            nc.sync.dma_start(ident_t[:], ident_d.ap())

            from contextlib import nullcontext
            with (tc.For_i(0, loop_n, 1) if loop_n > 1 else nullcontext()):
             for rep in range(repeat):
              for l in range(L):
                hsrc = xfull_d if (l == 0 or no_cc) else hf[l - 1]
                b1c = bias_t[:, 0 * L + l:0 * L + l + 1]
                b2c = bias_t[:, 1 * L + l:1 * L + l + 1]
                gac = bias_t[:, 2 * L + l:2 * L + l + 1]
                bec = bias_t[:, 3 * L + l:3 * L + l + 1]
                w1c = w1_t[:, l * F:(l + 1) * F]
                w2c = w2_t[:, l * F:(l + 1) * F]

                ssum = statp.tile([P, T], dt.float32, tag=f"ssum{l}")
                ssq = statp.tile([P, T], dt.float32, tag=f"ssq{l}")

                for t in range(T):
                    g_t = gpool.tile([P, 4 * BCPT, P], dt.float32, tag="G")
                    if no_gather:
                        nc.scalar.copy(g_t[:, 0, :], iota_t[:])
                    for b in range(4 if not no_gather else 0):
                        call = t * 4 + b
                        nc.gpsimd.dma_gather(
                            out_ap=g_t[:, b * BCPT:(b + 1) * BCPT, :],
                            in_ap=hsrc.ap()[b * cfg.BANK_ROWS:
                                            (b + 1) * cfg.BANK_ROWS, :],
                            idxs_ap=gidx_t[:, call * (CAPB // 16):
                                           (call + 1) * (CAPB // 16)],
                            num_idxs=CAPB,
                            num_idxs_reg=CAPB,
                            elem_size=F,
                            queue_num=b % num_swdge_queues,
                        )
                    agg_ps = psA.tile([P, P], dt.float32, tag="agg")
                    nch = 4 * BCPT
                    for ch in range(nch):
                        s_t = spool.tile([P, P], dt.float32, tag="S")
                        nc.vector.tensor_scalar(
                            out=s_t[:], in0=iota_t[:],
                            scalar1=drel_t[:, t * nch + ch:t * nch + ch + 1],
                            scalar2=None, op0=mybir.AluOpType.is_equal)
                        nc.tensor.matmul(agg_ps[:], lhsT=s_t[:],
                                         rhs=g_t[:, 0 if no_gather else ch, :],
                                         start=(ch == 0), stop=(ch == nch - 1))
                    # node-major agg -> SBUF -> transpose to feature-major
                    agg_nm = work.tile([P, P], dt.float32, tag="aggnm")
                    nc.scalar.copy(agg_nm[:], agg_ps[:])
                    aggT_ps = psT.tile([P, P], dt.float32, tag="aggT")
                    nc.tensor.transpose(aggT_ps[:], agg_nm[:], ident_t[:])
                    z1in = work.tile([P, P], dt.float32, tag="z1in")
                    nc.vector.tensor_add(z1in[:], h_loc[:, t * P:(t + 1) * P],
                                         aggT_ps[:])
                    # MLP (feature-major, weights stationary)
                    mp1 = psM.tile([P, P], dt.float32, tag="mp")
                    nc.tensor.matmul(mp1[:], lhsT=w1c, rhs=z1in[:],
                                     start=True, stop=True)
                    z1 = work.tile([P, P], dt.float32, tag="z1")
                    nc.scalar.activation(z1[:], mp1[:],
                                         mybir.ActivationFunctionType.Relu,
                                         bias=b1c)
                    mp2 = psM.tile([P, P], dt.float32, tag="mp")
                    nc.tensor.matmul(mp2[:], lhsT=w2c, rhs=z1[:],
                                     start=True, stop=True)
                    # z_pre overwrites h_loc tile in place
                    nc.scalar.activation(h_loc[:, t * P:(t + 1) * P], mp2[:],
                                         mybir.ActivationFunctionType.Relu,
                                         bias=b2c)
                    # zero phantom (unfilled + pad) seat columns, then stats
                    msk = spool.tile([P, P], dt.float32, tag="S")
                    nc.vector.tensor_scalar(
                        out=msk[:], in0=iota_t[:],
                        scalar1=filled_t[:, t:t + 1], scalar2=None,
                        op0=mybir.AluOpType.is_lt)
                    nc.vector.tensor_tensor(
                        out=h_loc[:, t * P:(t + 1) * P],
                        in0=h_loc[:, t * P:(t + 1) * P], in1=msk[:],
                        op=mybir.AluOpType.mult)
                    seat_ap = h_loc[:, t * P:(t + 1) * P]
                    nc.vector.tensor_reduce(
                        out=ssum[:, t:t + 1], in_=seat_ap,
                        axis=mybir.AxisListType.X, op=mybir.AluOpType.add)
                    sqs = work.tile([P, P], dt.float32, tag="sqs")
                    nc.scalar.activation(sqs[:], seat_ap,
                                         mybir.ActivationFunctionType.Square,
                                         accum_out=ssq[:, t:t + 1])

                # ---- BN stats allreduce ----
                red = work.tile([P, 2], dt.float32, tag="red")
                nc.vector.tensor_reduce(out=red[:, 0:1], in_=ssum[:],
                                        axis=mybir.AxisListType.X,
                                        op=mybir.AluOpType.add)
                nc.vector.tensor_reduce(out=red[:, 1:2], in_=ssq[:],
                                        axis=mybir.AxisListType.X,
                                        op=mybir.AluOpType.add)
                nc.sync.dma_start(st_in[l].ap(), red[:])
                if not no_cc:
                    nc.gpsimd.collective_compute(
                        "AllReduce", mybir.AluOpType.add, replica_groups=groups,
                        ins=[st_in[l].ap().opt()], outs=[st_out[l].ap().opt()])
                stt = work.tile([P, 2], dt.float32, tag="stt")
                nc.sync.dma_start(stt[:], (st_in[l] if no_cc else st_out[l]).ap())
                # mean, var, scale a, shift c
                mean = work.tile([P, 1], dt.float32, tag="mean")
                nc.vector.tensor_scalar(out=mean[:], in0=stt[:, 0:1],
                                        scalar1=inv_n, scalar2=None,
                                        op0=mybir.AluOpType.mult)
                var = work.tile([P, 1], dt.float32, tag="var")
                nc.vector.tensor_scalar(out=var[:], in0=stt[:, 1:2],
                                        scalar1=inv_n, scalar2=None,
                                        op0=mybir.AluOpType.mult)
                msq = work.tile([P, 1], dt.float32, tag="msq")
                nc.vector.tensor_tensor(out=msq[:], in0=mean[:], in1=mean[:],
                                        op=mybir.AluOpType.mult)
                nc.vector.tensor_tensor(out=var[:], in0=var[:], in1=msq[:],
                                        op=mybir.AluOpType.subtract)
                nc.vector.tensor_scalar(out=var[:], in0=var[:],
                                        scalar1=BN_EPS, scalar2=None,
                                        op0=mybir.AluOpType.add)
                sd = work.tile([P, 1], dt.float32, tag="sd")
                nc.scalar.activation(sd[:], var[:],
                                     mybir.ActivationFunctionType.Sqrt)
                inv = work.tile([P, 1], dt.float32, tag="inv")
                nc.vector.reciprocal(inv[:], sd[:])
                a_t = work.tile([P, 1], dt.float32, tag="a")
                nc.vector.tensor_tensor(out=a_t[:], in0=inv[:], in1=gac,
                                        op=mybir.AluOpType.mult)
                c_t = work.tile([P, 1], dt.float32, tag="c")
                nc.vector.tensor_tensor(out=c_t[:], in0=mean[:], in1=a_t[:],
                                        op=mybir.AluOpType.mult)
                nc.vector.tensor_tensor(out=c_t[:], in0=bec, in1=c_t[:],
                                        op=mybir.AluOpType.subtract)

                # ---- normalize in place, transpose, pool, store ----
                pool_ps = psP.tile([P, P], dt.float32, tag="pool")
                for t in range(T):
                    nc.scalar.activation(
                        h_loc[:, t * P:(t + 1) * P],
                        h_loc[:, t * P:(t + 1) * P],
                        mybir.ActivationFunctionType.Identity,
                        bias=c_t[:, 0:1], scale=a_t[:, 0:1])
                    zT_ps = psT.tile([P, P], dt.float32, tag="aggT")
                    nc.tensor.transpose(zT_ps[:], h_loc[:, t * P:(t + 1) * P],
                                        ident_t[:])
                    znm = work.tile([P, P], dt.float32, tag="znm")
                    nc.scalar.copy(znm[:], zT_ps[:])
                    if l < L - 1:
                        nc.sync.dma_start(zz[l].ap()[t * P:(t + 1) * P, :],
                                          znm[:])
                    sb_t = spool.tile([P, P], dt.float32, tag="S")
                    nc.vector.tensor_scalar(
                        out=sb_t[:], in0=iota_t[:],
                        scalar1=brel_t[:, t:t + 1], scalar2=None,
                        op0=mybir.AluOpType.is_equal)
                    nc.tensor.matmul(pool_ps[:], lhsT=sb_t[:], rhs=znm[:],
                                     start=(t == 0), stop=(t == T - 1),
                                     skip_group_check=True)
                poolsb = work.tile([P, P], dt.float32, tag="poolsb")
                nc.scalar.copy(poolsb[:], pool_ps[:])
                nc.sync.dma_start(pools_d.ap()[:, l * F:(l + 1) * F],
                                  poolsb[:])

                if l < L - 1 and not no_cc:
                    nc.gpsimd.collective_compute(
                        "AllGather", mybir.AluOpType.bypass,
                        replica_groups=groups,
                        ins=[zz[l].ap().opt()], outs=[hf[l].ap().opt()])

    nc.compile()
    return nc


def make_in_maps(per_core, x_full, inputs):
    W1 = np.asarray(inputs['W1'], np.float32)
    W2 = np.asarray(inputs['W2'], np.float32)
    b1 = np.asarray(inputs['b1'], np.float32)
    b2 = np.asarray(inputs['b2'], np.float32)
    gamma = np.asarray(inputs['gamma'], np.float32)
    beta = np.asarray(inputs['beta'], np.float32)
    w1_h = np.ascontiguousarray(np.concatenate([W1[i] for i in range(L)], 1))
    w2_h = np.ascontiguousarray(np.concatenate([W2[i] for i in range(L)], 1))
    bias_h = np.ascontiguousarray(
        np.concatenate([b1.T, b2.T, gamma.T, beta.T], 1))
    iota_h = np.tile(np.arange(P, dtype=np.float32), (P, 1))
    ident_h = np.eye(P, dtype=np.float32)
    in_maps = []
    for c in range(NCORES):
        pc = per_core[c]
        in_maps.append({
            "xT": pc["xT"], "xnm": pc["xnm"], "gidx": pc["gidx"],
            "dstrel": pc["dstrel"], "brel": pc["batch_rel"],
            "filled": pc["filled"],
            "w1": w1_h, "w2": w2_h, "bias": bias_h,
            "iotat": iota_h, "ident": ident_h,
        })
    return in_maps


def build_null_kernel(cfg, BCPT):
    """Same I/O signature as the real kernel, trivial body. Used to measure
    the non-compute overhead (upload/dispatch) of a run for timing deltas."""
    CAPB = BCPT * 128
    T = cfg.T
    dt = mybir.dt
    nc = bacc.Bacc("TRN2", target_bir_lowering=False, debug=False,
                   enable_asserts=True, num_devices=NCORES,
                   num_swdge_queues=4)
    xT_d = nc.dram_tensor("xT", [P, cfg.COLS], dt.float32,
                          kind="ExternalInput")
    nc.dram_tensor("xnm", [cfg.COLS, F], dt.float32, kind="ExternalInput")
    nc.dram_tensor("gidx", [P, T * 4 * (CAPB // 16)], dt.int16,
                   kind="ExternalInput")
    nc.dram_tensor("dstrel", [P, T * 4 * BCPT], dt.float32,
                   kind="ExternalInput")
    nc.dram_tensor("brel", [P, T], dt.float32, kind="ExternalInput")
    nc.dram_tensor("filled", [P, T], dt.float32, kind="ExternalInput")
    nc.dram_tensor("w1", [P, L * F], dt.float32, kind="ExternalInput")
    nc.dram_tensor("w2", [P, L * F], dt.float32, kind="ExternalInput")
    nc.dram_tensor("bias", [P, 4 * L], dt.float32, kind="ExternalInput")
    nc.dram_tensor("iotat", [P, P], dt.float32, kind="ExternalInput")
    nc.dram_tensor("ident", [P, P], dt.float32, kind="ExternalInput")
    pools_d = nc.dram_tensor("pools", [P, L * F], dt.float32,
                             kind="ExternalOutput")
    with tile.TileContext(nc) as tc:
        with tc.tile_pool(name="sb", bufs=1) as sb:
            t0 = sb.tile([P, L * F], dt.float32)
            nc.sync.dma_start(t0[:], xT_d.ap()[:, 0:L * F])
            nc.sync.dma_start(pools_d.ap(), t0[:])
    nc.compile()
    return nc


# ---------------- entry point ----------------
def kernel(x, edge_index, batch, W1, b1, W2, b2, gamma, beta):
    cfg = FULL
    x = np.asarray(x, np.float32)
    edge_index = np.asarray(edge_index, np.int32)
    batch = np.asarray(batch, np.int32)
    W1 = np.asarray(W1, np.float32)
    b1 = np.asarray(b1, np.float32)
    W2 = np.asarray(W2, np.float32)
    b2 = np.asarray(b2, np.float32)
    gamma = np.asarray(gamma, np.float32)
    beta = np.asarray(beta, np.float32)

    x_full, per_core, meta = preprocess(cfg, x, edge_index, batch)
    nc = build_kernel(cfg, meta['BCPT'])
    in_maps = make_in_maps(per_core, x_full, dict(
        W1=W1, W2=W2, b1=b1, b2=b2, gamma=gamma, beta=beta))

    import time as _time
    last_exc = None
    for attempt in range(3):
        try:
            res = bass_utils.run_bass_kernel_spmd(
                nc, in_maps, core_ids=list(range(NCORES)))
            break
        except Exception as e:       # transient device wedge -> retry
            last_exc = e
            _time.sleep(20)
    else:
        raise last_exc

    out = np.zeros((NUM_GRAPHS, L * F), np.float32)
    for c in range(NCORES):
        pool_c = res.results[c]["pools"]          # [128, L*F]
        g0, sp = meta['g_bases'][c], meta['spans'][c]
        out[g0:g0 + sp] += pool_c[:sp]
    return out


if __name__ == "__main__":
    import reference
    inputs = reference.setup_inputs()
    inputs = {k: np.asarray(v) for k, v in inputs.items()}
    got = kernel(**inputs)
    print("kernel output shape:", got.shape)



# revision 23
# speedup vs baseline: 45.1753x; 1.2054x over previous
"""GIN encoder (3x GINConv+BN + per-layer global_add_pool) on 8 Trainium2 cores.

Strategy:
  - Nodes sharded round 8 cores (12500 each). Each core's nodes are permuted
    into 102 "tiles" of 125 seats (+3 pad) chosen to balance gather traffic
    across 4 source banks (degree-sorted round-robin + local repair).
  - Edges sharded by dst owner; per (dst-tile, src-bank) edge slots padded to
    a fixed cap -> fully uniform SPMD program.
  - Aggregation h[src] gather via dma_gather (4 SWDGE queues) from a
    [104448, 128] node-feature table (AllGather of per-core node-major
    slices for layer 0, AllGather outputs for layers 1, 2), then
    segment-sum as one-hot matmuls accumulating feature-major in PSUM.
  - MLP feature-major on PE (weights stationary), BN stats via [128,2]
    AllReduce, pool via one-hot matmul; host sums per-core partial pools.
  - All per-tile pipelines run inside For_i hardware loops with
    register-indexed (DynSlice) APs: ~1.1k static instructions vs ~24k
    fully unrolled. This runtime charges ~130us per static instruction
    per execution, so static code size dominates device time; hardware
    loop re-execution is nearly free (measured ~10ns/instruction).
"""
import sys
sys.path.insert(0, '/opt/trn_rl_repo')

import numpy as np

import concourse.bass as bass
import concourse.tile as tile
from concourse import bacc, mybir, library_config
from concourse import bass_utils

# ---------------- problem config (hardcoded from spec) ----------------
NCORES = 8
N = 100000
F = 128
E = 1600000
L = 3
NUM_GRAPHS = 512
BN_EPS = 1e-5
P = 128


class Cfg:
    def __init__(self, N, E, num_graphs, T, seats, capb):
        self.N = N
        self.E = E
        self.num_graphs = num_graphs
        self.NPC = N // NCORES            # nodes per core
        self.T = T                        # dst tiles per core
        self.SEATS = seats                # real seats per tile (<=128)
        self.CAPB = capb                  # slots per (tile, bank); %128==0
        self.BCPT = capb // 128           # chunks per (tile, bank)
        self.COLS = T * P                 # padded node columns per core
        self.HF_ROWS = NCORES * self.COLS
        self.NBANK = 4
        self.BANK_ROWS = self.HF_ROWS // 4
        assert self.BANK_ROWS < 32767
        assert self.T * self.SEATS >= self.NPC
        assert self.CAPB <= 1024 and self.CAPB % 128 == 0


FULL = Cfg(N, E, NUM_GRAPHS, T=102, seats=125, capb=512)


# ---------------- host-side preprocessing ----------------
def preprocess(cfg, x, edge_index, batch):
    """Returns per-core input dicts + metadata for output assembly."""
    NPC, T, SEATS = cfg.NPC, cfg.T, cfg.SEATS
    src = edge_index[0].astype(np.int64)
    dst = edge_index[1].astype(np.int64)
    owner = dst // NPC
    src_bank = src // (2 * NPC)          # = owner(src)//2, permutation-invariant

    # --- per-core tile assignment balancing per-bank degree ---
    tile_of_g = np.empty(cfg.N, np.int64)     # local tile of each node
    seat_of_g = np.empty(cfg.N, np.int64)     # seat within tile
    capb_used = cfg.CAPB
    for c in range(NCORES):
        m = owner == c
        d_loc = dst[m] - c * NPC
        counts = np.zeros((NPC, 4), np.int64)
        np.add.at(counts, (d_loc, src_bank[m]), 1)
        deg = counts.sum(1)
        order = np.argsort(-deg, kind='stable')

        # fast path: degree-sorted round-robin + local repair of cap
        # violations; falls back to the exact greedy below if repair fails
        ok_all = False
        tile_of = np.empty(NPC, np.int64)
        tile_of[order] = np.arange(NPC) % T
        loads = np.zeros((T, 4), np.int64)
        for b in range(4):
            loads[:, b] = np.bincount(tile_of, weights=counts[:, b],
                                      minlength=T).astype(np.int64)
        nseat = np.bincount(tile_of, minlength=T)
        if nseat.max() <= SEATS:
            for _repair in range(200):
                over = np.argwhere(loads > cfg.CAPB)
                if len(over) == 0:
                    ok_all = True
                    break
                t_bad, b_bad = over[0]
                cand = np.where(tile_of == t_bad)[0]
                cand = cand[np.argsort(-counts[cand, b_bad])]
                moved = False
                for d in cand:
                    room = ((loads + counts[d] <= cfg.CAPB).all(1)
                            & (nseat < SEATS))
                    room[t_bad] = False
                    tgt = np.argwhere(room)
                    if len(tgt):
                        t_new = int(tgt[0][0])
                        loads[t_bad] -= counts[d]
                        loads[t_new] += counts[d]
                        nseat[t_bad] -= 1
                        nseat[t_new] += 1
                        tile_of[d] = t_new
                        moved = True
                        break
                if not moved:
                    break
        if ok_all:
            seat_of = np.empty(NPC, np.int64)
            order_t = np.argsort(tile_of, kind='stable')
            tt = tile_of[order_t]
            starts = np.searchsorted(tt, np.arange(T))
            seat_of[order_t] = np.arange(NPC) - np.repeat(
                starts, np.bincount(tt, minlength=T))
            tile_of_g[c * NPC:(c + 1) * NPC] = tile_of
            seat_of_g[c * NPC:(c + 1) * NPC] = seat_of
            continue

        for cap_try in (cfg.CAPB, cfg.CAPB + 128, cfg.CAPB + 256, 1024):
            rem = np.full((T, 4), cap_try, np.int64)
            seats = np.full(T, SEATS, np.int64)
            tile_of = np.full(NPC, -1, np.int64)
            seat_of = np.full(NPC, -1, np.int64)
            ok_all = True
            for d in order:
                v = counts[d]
                feas = (seats > 0) & (rem >= v).all(1)
                if not feas.any():
                    ok_all = False
                    break
                slack = (rem - v).min(1).astype(np.float64)
                slack[~feas] = -1e18
                t = int(np.argmax(slack))
                tile_of[d] = t
                seat_of[d] = SEATS - seats[t]
                rem[t] -= v
                seats[t] -= 1
            if ok_all:
                capb_used = max(capb_used, cap_try)
                break
        assert ok_all, "tile assignment failed even at cap 1024"
        tile_of_g[c * NPC:(c + 1) * NPC] = tile_of
        seat_of_g[c * NPC:(c + 1) * NPC] = seat_of

    CAPB = capb_used
    BCPT = CAPB // 128
    perm_pos = tile_of_g * P + seat_of_g                     # within-core col
    perm_row = (np.arange(cfg.N) // NPC) * cfg.COLS + perm_pos  # global row

    # --- x tables ---
    xf = np.asarray(x, np.float32)
    x_full = np.zeros((cfg.HF_ROWS, F), np.float32)
    x_full[perm_row] = xf
    g_bases, spans = [], []
    per_core = []
    for c in range(NCORES):
        lo, hi = c * NPC, (c + 1) * NPC
        xnm = np.zeros((cfg.COLS, F), np.float32)            # node-major
        xnm[perm_pos[lo:hi]] = xf[lo:hi]
        xlT = np.ascontiguousarray(xnm.T)                    # [128, COLS]

        g_base = int(batch[lo])
        span = int(batch[hi - 1]) - g_base + 1
        assert span <= P, f"graph span {span} exceeds 128"
        g_bases.append(g_base)
        spans.append(span)
        br = np.full(cfg.COLS, -1.0, np.float32)
        br[perm_pos[lo:hi]] = (batch[lo:hi] - g_base).astype(np.float32)
        batch_rel = np.ascontiguousarray(br.reshape(T, P).T)  # [128, T]
        filled_row = np.bincount(tile_of_g[lo:hi], minlength=T).astype(np.float32)
        filled_tbl = np.tile(filled_row, (P, 1))              # [128, T]

        # --- edge slot tables ---
        m = owner == c
        e_src = src[m]
        e_dst = dst[m] - c * NPC
        key = tile_of_g[c * NPC + e_dst] * 4 + src_bank[m]
        order_e = np.argsort(key, kind='stable')
        key_s = key[order_e]
        cnt = np.bincount(key_s, minlength=T * 4)
        assert cnt.max() <= CAPB, f"(tile,bank) count {cnt.max()} > cap {CAPB}"
        cstart = np.zeros(T * 4, np.int64)
        cstart[1:] = np.cumsum(cnt)[:-1]
        within = np.arange(len(key_s)) - np.repeat(cstart, cnt)
        slot = key_s * CAPB + within
        gidx_flat = np.zeros(T * 4 * CAPB, np.int64)
        drel_flat = np.full(T * 4 * CAPB, -1.0, np.float32)
        gidx_flat[slot] = perm_row[e_src[order_e]] % cfg.BANK_ROWS
        drel_flat[slot] = seat_of_g[c * NPC + e_dst[order_e]]
        # idx wrap: per call (t,b): CAPB values -> [16, CAPB//16], tile x8
        w = gidx_flat.reshape(T * 4, CAPB // 16, 16).transpose(2, 0, 1)
        w = w.reshape(16, T * 4 * (CAPB // 16))
        gidx_h = np.tile(w, (8, 1)).astype(np.int16)         # [128, cols]
        drel_h = np.ascontiguousarray(
            drel_flat.reshape(T * 4 * BCPT, P).T)             # [128, chunks]

        per_core.append(dict(xT=xlT, xnm=xnm, batch_rel=batch_rel,
                             gidx=gidx_h, dstrel=drel_h, filled=filled_tbl))

    meta = dict(CAPB=CAPB, BCPT=BCPT, g_bases=g_bases, spans=spans)
    return x_full, per_core, meta


# ---------------- device kernel builder (v2: hardware loops) ----------------
def build_kernel_v2(cfg, BCPT, num_swdge_queues=4, repeat=1, no_cc=False,
                    fat_onehot=True, t_frac=1.0):
    """Same algorithm as v1 but the per-tile pipelines run inside For_i
    hardware loops with register-indexed APs. Static instruction count
    ~220 vs ~15.6k; this runtime charges ~127us per STATIC instruction,
    so this is the dominant win."""
    from concourse.bass import ds
    CAPB = BCPT * 128
    T = cfg.T
    nch = 4 * BCPT
    C16 = CAPB // 16
    dt = mybir.dt
    nc = bacc.Bacc("TRN2", target_bir_lowering=False, debug=False,
                   enable_asserts=True, num_devices=NCORES,
                   num_swdge_queues=num_swdge_queues)

    # inputs (v1 signature minus the replicated xfull table: the layer-0
    # gather table is AllGathered on device from the per-core xT slices)
    xT_d = nc.dram_tensor("xT", [P, cfg.COLS], dt.float32, kind="ExternalInput")
    gidx_d = nc.dram_tensor("gidx", [P, T * 4 * C16], dt.int16,
                            kind="ExternalInput")
    dstrel_d = nc.dram_tensor("dstrel", [P, T * nch], dt.float32,
                              kind="ExternalInput")
    xnm_in = nc.dram_tensor("xnm", [cfg.COLS, F], dt.float32,
                            kind="ExternalInput")
    xnm_d = nc.dram_tensor("xnmi", [cfg.COLS, F], dt.float32, kind="Internal")
    xg_d = nc.dram_tensor("xg", [cfg.HF_ROWS, F], dt.float32,
                          kind="Internal", addr_space="Shared")
    brel_d = nc.dram_tensor("brel", [P, T], dt.float32, kind="ExternalInput")
    filled_d = nc.dram_tensor("filled", [P, T], dt.float32, kind="ExternalInput")
    w1_d = nc.dram_tensor("w1", [P, L * F], dt.float32, kind="ExternalInput")
    w2_d = nc.dram_tensor("w2", [P, L * F], dt.float32, kind="ExternalInput")
    bias_d = nc.dram_tensor("bias", [P, 4 * L], dt.float32, kind="ExternalInput")
    iota_d = nc.dram_tensor("iotat", [P, P], dt.float32, kind="ExternalInput")
    ident_d = nc.dram_tensor("ident", [P, P], dt.float32, kind="ExternalInput")
    pools_d = nc.dram_tensor("pools", [P, L * F], dt.float32,
                             kind="ExternalOutput")

    zz = [nc.dram_tensor(f"zz{l}", [cfg.COLS, F], dt.float32, kind="Internal")
          for l in range(L - 1)]
    hf = [nc.dram_tensor(f"hf{l}", [cfg.HF_ROWS, F], dt.float32,
                         kind="Internal", addr_space="Shared")
          for l in range(L - 1)]
    st_in = [nc.dram_tensor(f"stin{l}", [P, 2], dt.float32, kind="Internal")
             for l in range(L)]
    st_out = [nc.dram_tensor(f"stout{l}", [P, 2], dt.float32, kind="Internal",
                             addr_space="Shared") for l in range(L)]

    inv_n = 1.0 / cfg.N
    groups = [list(range(NCORES))]

    with tile.TileContext(nc) as tc:
        with tc.tile_pool(name="big", bufs=1) as big, \
             tc.tile_pool(name="gpool", bufs=2) as gpool, \
             tc.tile_pool(name="spool", bufs=2) as spool, \
             tc.tile_pool(name="work", bufs=2) as work, \
             tc.tile_pool(name="stat", bufs=1) as statp, \
             tc.tile_pool(name="psA", bufs=2, space="PSUM") as psA, \
             tc.tile_pool(name="psT", bufs=2, space="PSUM") as psT, \
             tc.tile_pool(name="psM", bufs=2, space="PSUM") as psM, \
             tc.tile_pool(name="psP", bufs=2, space="PSUM") as psP:

            nc.gpsimd.load_library(library_config.mlp)

            h_loc = big.tile([P, cfg.COLS], dt.float32)
            nc.sync.dma_start(h_loc[:], xT_d.ap())
            gidx_t = big.tile([P, T * 4 * C16], dt.int16)
            nc.sync.dma_start(gidx_t[:], gidx_d.ap())
            drel_t = big.tile([P, T * nch], dt.float32)
            nc.sync.dma_start(drel_t[:], dstrel_d.ap())
            brel_t = big.tile([P, T], dt.float32)
            nc.sync.dma_start(brel_t[:], brel_d.ap())
            filled_t = big.tile([P, T], dt.float32)
            nc.sync.dma_start(filled_t[:], filled_d.ap())
            w1_t = big.tile([P, L * F], dt.float32)
            nc.sync.dma_start(w1_t[:], w1_d.ap())
            w2_t = big.tile([P, L * F], dt.float32)
            nc.sync.dma_start(w2_t[:], w2_d.ap())
            bias_t = big.tile([P, 4 * L], dt.float32)
            nc.sync.dma_start(bias_t[:], bias_d.ap())
            iota_t = big.tile([P, P], dt.float32)
            nc.sync.dma_start(iota_t[:], iota_d.ap())
            ident_t = big.tile([P, P], dt.float32)
            nc.sync.dma_start(ident_t[:], ident_d.ap())

            # layer-0 gather table: AllGather the host-provided node-major x
            # (collective ins must be internal DRAM -> one local copy first)
            nc.sync.dma_start(xnm_d.ap(), xnm_in.ap())
            if not no_cc:
                nc.gpsimd.collective_compute(
                    "AllGather", mybir.AluOpType.bypass,
                    replica_groups=groups,
                    ins=[xnm_d.ap().opt()], outs=[xg_d.ap().opt()])

            for rep in range(repeat):
             for l in range(L):
                hsrc = xg_d if (l == 0 or no_cc) else hf[l - 1]
                b1c = bias_t[:, 0 * L + l:0 * L + l + 1]
                b2c = bias_t[:, 1 * L + l:1 * L + l + 1]
                gac = bias_t[:, 2 * L + l:2 * L + l + 1]
                bec = bias_t[:, 3 * L + l:3 * L + l + 1]
                w1c = w1_t[:, l * F:(l + 1) * F]
                w2c = w2_t[:, l * F:(l + 1) * F]

                acc = statp.tile([P, 2], dt.float32, tag=f"acc{l}{rep}")
                nc.vector.memset(acc[:], 0.0)

                # ---- phase A: aggregate + MLP, HW loop over dst tiles ----
                with tc.For_i(0, max(1, int(T * t_frac)), 1) as t:
                    g_t = gpool.tile([P, nch, P], dt.float32, tag="G")
                    for b in range(4):
                        nc.gpsimd.dma_gather(
                            out_ap=g_t[:, b * BCPT:(b + 1) * BCPT, :],
                            in_ap=hsrc.ap()[b * cfg.BANK_ROWS:
                                            (b + 1) * cfg.BANK_ROWS, :],
                            idxs_ap=gidx_t[:, ds(t * (4 * C16) + b * C16, C16)],
                            num_idxs=CAPB,
                            num_idxs_reg=CAPB,
                            elem_size=F,
                            queue_num=b % num_swdge_queues,
                        )
                    # one-hot seat matrices for all chunks
                    if fat_onehot:
                        s_all = spool.tile([P, nch, P], dt.float32, tag="S")
                        nc.vector.tensor_tensor(
                            out=s_all[:],
                            in0=iota_t[:].unsqueeze(1)
                                .broadcast_to([P, nch, P]),
                            in1=drel_t[:, ds(t * nch, nch)].unsqueeze(2)
                                .broadcast_to([P, nch, P]),
                            op=mybir.AluOpType.is_equal)
                    else:
                        s_all = spool.tile([P, nch, P], dt.float32, tag="S")
                        for ch in range(nch):
                            nc.vector.tensor_scalar(
                                out=s_all[:, ch, :], in0=iota_t[:],
                                scalar1=drel_t[:, ds(t * nch + ch, 1)],
                                scalar2=None, op0=mybir.AluOpType.is_equal)
                    # feature-major agg: agg[f, seat] += g[e,f]^T @ onehot[e,seat]
                    agg_ps = psA.tile([P, P], dt.float32, tag="agg")
                    for ch in range(nch):
                        nc.tensor.matmul(agg_ps[:], lhsT=g_t[:, ch, :],
                                         rhs=s_all[:, ch, :],
                                         start=(ch == 0), stop=(ch == nch - 1))
                    z1in = work.tile([P, P], dt.float32, tag="z1in")
                    nc.vector.tensor_add(z1in[:], h_loc[:, ds(t * P, P)],
                                         agg_ps[:])
                    mp1 = psM.tile([P, P], dt.float32, tag="mp")
                    nc.tensor.matmul(mp1[:], lhsT=w1c, rhs=z1in[:],
                                     start=True, stop=True)
                    z1 = work.tile([P, P], dt.float32, tag="z1")
                    nc.scalar.activation(z1[:], mp1[:],
                                         mybir.ActivationFunctionType.Relu,
                                         bias=b1c)
                    mp2 = psM.tile([P, P], dt.float32, tag="mp")
                    nc.tensor.matmul(mp2[:], lhsT=w2c, rhs=z1[:],
                                     start=True, stop=True)
                    nc.scalar.activation(h_loc[:, ds(t * P, P)], mp2[:],
                                         mybir.ActivationFunctionType.Relu,
                                         bias=b2c)
                    msk = work.tile([P, P], dt.float32, tag="msk")
                    nc.vector.tensor_scalar(
                        out=msk[:], in0=iota_t[:],
                        scalar1=filled_t[:, ds(t, 1)], scalar2=None,
                        op0=mybir.AluOpType.is_lt)
                    nc.vector.tensor_tensor(
                        out=h_loc[:, ds(t * P, P)],
                        in0=h_loc[:, ds(t * P, P)], in1=msk[:],
                        op=mybir.AluOpType.mult)
                    s12 = work.tile([P, 2], dt.float32, tag="s12")
                    nc.vector.tensor_reduce(
                        out=s12[:, 0:1], in_=h_loc[:, ds(t * P, P)],
                        axis=mybir.AxisListType.X, op=mybir.AluOpType.add)
                    sqs = work.tile([P, P], dt.float32, tag="sqs")
                    nc.scalar.activation(sqs[:], h_loc[:, ds(t * P, P)],
                                         mybir.ActivationFunctionType.Square,
                                         accum_out=s12[:, 1:2])
                    nc.vector.tensor_tensor(out=acc[:], in0=acc[:],
                                            in1=s12[:],
                                            op=mybir.AluOpType.add)

                # ---- BN stats allreduce ----
                nc.sync.dma_start(st_in[l].ap(), acc[:])
                if not no_cc:
                    nc.gpsimd.collective_compute(
                        "AllReduce", mybir.AluOpType.add, replica_groups=groups,
                        ins=[st_in[l].ap().opt()], outs=[st_out[l].ap().opt()])
                stt = work.tile([P, 2], dt.float32, tag="stt")
                nc.sync.dma_start(stt[:], (st_in[l] if no_cc else st_out[l]).ap())
                mean = work.tile([P, 1], dt.float32, tag="mean")
                nc.vector.tensor_scalar(out=mean[:], in0=stt[:, 0:1],
                                        scalar1=inv_n, scalar2=None,
                                        op0=mybir.AluOpType.mult)
                var = work.tile([P, 1], dt.float32, tag="var")
                nc.vector.tensor_scalar(out=var[:], in0=stt[:, 1:2],
                                        scalar1=inv_n, scalar2=None,
                                        op0=mybir.AluOpType.mult)
                msq = work.tile([P, 1], dt.float32, tag="msq")
                nc.vector.tensor_tensor(out=msq[:], in0=mean[:], in1=mean[:],
                                        op=mybir.AluOpType.mult)
                nc.vector.tensor_tensor(out=var[:], in0=var[:], in1=msq[:],
                                        op=mybir.AluOpType.subtract)
                nc.vector.tensor_scalar(out=var[:], in0=var[:],
                                        scalar1=BN_EPS, scalar2=None,
                                        op0=mybir.AluOpType.add)
                sd = work.tile([P, 1], dt.float32, tag="sd")
                nc.scalar.activation(sd[:], var[:],
                                     mybir.ActivationFunctionType.Sqrt)
                inv = work.tile([P, 1], dt.float32, tag="inv")
                nc.vector.reciprocal(inv[:], sd[:])
                a_t = work.tile([P, 1], dt.float32, tag="a")
                nc.vector.tensor_tensor(out=a_t[:], in0=inv[:], in1=gac,
                                        op=mybir.AluOpType.mult)
                c_t = work.tile([P, 1], dt.float32, tag="c")
                nc.vector.tensor_tensor(out=c_t[:], in0=mean[:], in1=a_t[:],
                                        op=mybir.AluOpType.mult)
                nc.vector.tensor_tensor(out=c_t[:], in0=bec, in1=c_t[:],
                                        op=mybir.AluOpType.subtract)

                # ---- phase C: normalize, transpose, pool (HW loop) ----
                pool_sb = statp.tile([P, P], dt.float32, tag=f"pool{l}{rep}")
                nc.vector.memset(pool_sb[:], 0.0)
                with tc.For_i(0, max(1, int(T * t_frac)), 1) as t:
                    # walrus matmul lhsT needs a static offset: stage the
                    # normalized tile in a fixed SBUF buffer for the transpose
                    znf = work.tile([P, P], dt.float32, tag="znf")
                    if l < L - 1:
                        nc.scalar.activation(
                            h_loc[:, ds(t * P, P)], h_loc[:, ds(t * P, P)],
                            mybir.ActivationFunctionType.Identity,
                            bias=c_t[:, 0:1], scale=a_t[:, 0:1])
                        nc.scalar.copy(znf[:], h_loc[:, ds(t * P, P)])
                    else:
                        nc.scalar.activation(
                            znf[:], h_loc[:, ds(t * P, P)],
                            mybir.ActivationFunctionType.Identity,
                            bias=c_t[:, 0:1], scale=a_t[:, 0:1])
                    zT_ps = psT.tile([P, P], dt.float32, tag="zT")
                    nc.tensor.transpose(zT_ps[:], znf[:], ident_t[:])
                    znm = work.tile([P, P], dt.float32, tag="znm")
                    nc.scalar.copy(znm[:], zT_ps[:])
                    if l < L - 1:
                        nc.sync.dma_start(zz[l].ap()[ds(t * P, P), :], znm[:])
                    sb_t = spool.tile([P, P], dt.float32, tag="SB")
                    nc.vector.tensor_scalar(
                        out=sb_t[:], in0=iota_t[:],
                        scalar1=brel_t[:, ds(t, 1)], scalar2=None,
                        op0=mybir.AluOpType.is_equal)
                    pp = psP.tile([P, P], dt.float32, tag="pp")
                    nc.tensor.matmul(pp[:], lhsT=sb_t[:], rhs=znm[:],
                                     start=True, stop=True)
                    nc.vector.tensor_add(pool_sb[:], pool_sb[:], pp[:])
                nc.sync.dma_start(pools_d.ap()[:, l * F:(l + 1) * F],
                                  pool_sb[:])

                if l < L - 1 and not no_cc:
                    nc.gpsimd.collective_compute(
                        "AllGather", mybir.AluOpType.bypass,
                        replica_groups=groups,
                        ins=[zz[l].ap().opt()], outs=[hf[l].ap().opt()])

    nc.compile()
    return nc


def build_kernel(cfg, BCPT, **kw):
    return build_kernel_v2(cfg, BCPT, **kw)


# ---------------- v1 (fully unrolled; kept for reference/fallback) ----------
def build_kernel_v1(cfg, BCPT, num_swdge_queues=4, repeat=1, loop_n=1, no_cc=False, no_gather=False):
    CAPB = BCPT * 128
    T = cfg.T
    dt = mybir.dt
    nc = bacc.Bacc("TRN2", target_bir_lowering=False, debug=False,
                   enable_asserts=True, num_devices=NCORES,
                   num_swdge_queues=num_swdge_queues)

    # inputs
    xT_d = nc.dram_tensor("xT", [P, cfg.COLS], dt.float32, kind="ExternalInput")
    xfull_d = nc.dram_tensor("xfull", [cfg.HF_ROWS, F], dt.float32,
                             kind="ExternalInput")
    gidx_d = nc.dram_tensor("gidx", [P, T * 4 * (CAPB // 16)], dt.int16,
                            kind="ExternalInput")
    dstrel_d = nc.dram_tensor("dstrel", [P, T * 4 * BCPT], dt.float32,
                              kind="ExternalInput")
    brel_d = nc.dram_tensor("brel", [P, T], dt.float32, kind="ExternalInput")
    filled_d = nc.dram_tensor("filled", [P, T], dt.float32, kind="ExternalInput")
    w1_d = nc.dram_tensor("w1", [P, L * F], dt.float32, kind="ExternalInput")
    w2_d = nc.dram_tensor("w2", [P, L * F], dt.float32, kind="ExternalInput")
    bias_d = nc.dram_tensor("bias", [P, 4 * L], dt.float32, kind="ExternalInput")
    iota_d = nc.dram_tensor("iotat", [P, P], dt.float32, kind="ExternalInput")
    ident_d = nc.dram_tensor("ident", [P, P], dt.float32, kind="ExternalInput")
    pools_d = nc.dram_tensor("pools", [P, L * F], dt.float32,
                             kind="ExternalOutput")

    # internal DRAM
    zz = [nc.dram_tensor(f"zz{l}", [cfg.COLS, F], dt.float32, kind="Internal")
          for l in range(L - 1)]
    hf = [nc.dram_tensor(f"hf{l}", [cfg.HF_ROWS, F], dt.float32,
                         kind="Internal", addr_space="Shared")
          for l in range(L - 1)]
    st_in = [nc.dram_tensor(f"stin{l}", [P, 2], dt.float32, kind="Internal")
             for l in range(L)]
    st_out = [nc.dram_tensor(f"stout{l}", [P, 2], dt.float32, kind="Internal",
                             addr_space="Shared") for l in range(L)]

    inv_n = 1.0 / cfg.N
    groups = [list(range(NCORES))]

    with tile.TileContext(nc) as tc:
        with tc.tile_pool(name="big", bufs=1) as big, \
             tc.tile_pool(name="gpool", bufs=3) as gpool, \
             tc.tile_pool(name="spool", bufs=8) as spool, \
             tc.tile_pool(name="work", bufs=4) as work, \
             tc.tile_pool(name="stat", bufs=1) as statp, \
             tc.tile_pool(name="psA", bufs=2, space="PSUM") as psA, \
             tc.tile_pool(name="psT", bufs=2, space="PSUM") as psT, \
             tc.tile_pool(name="psM", bufs=2, space="PSUM") as psM, \
             tc.tile_pool(name="psP", bufs=1, space="PSUM") as psP:

            nc.gpsimd.load_library(library_config.mlp)

            h_loc = big.tile([P, cfg.COLS], dt.float32)       # feature-major h
            nc.sync.dma_start(h_loc[:], xT_d.ap())
            gidx_t = big.tile([P, T * 4 * (CAPB // 16)], dt.int16)
            nc.sync.dma_start(gidx_t[:], gidx_d.ap())
            drel_t = big.tile([P, T * 4 * BCPT], dt.float32)
            nc.sync.dma_start(drel_t[:], dstrel_d.ap())
            brel_t = big.tile([P, T], dt.float32)
            nc.sync.dma_start(brel_t[:], brel_d.ap())
            filled_t = big.tile([P, T], dt.float32)
            nc.sync.dma_start(filled_t[:], filled_d.ap())
            w1_t = big.tile([P, L * F], dt.float32)
            nc.sync.dma_start(w1_t[:], w1_d.ap())
            w2_t = big.tile([P, L * F], dt.float32)
            nc.sync.dma_start(w2_t[:], w2_d.ap())
            bias_t = big.tile([P, 4 * L], dt.float32)
            nc.sync.dma_start(bias_t[:], bias_d.ap())
            iota_t = big.tile([P, P], dt.float32)
            nc.sync.dma_start(iota_t[:], iota_d.ap())
            ident_t = big.tile([P, P], dt.float32)
            nc.sync.dma_start(ident_t[:], ident_d.ap())

            from contextlib import nullcontext
            with (tc.For_i(0, loop_n, 1) if loop_n > 1 else nullcontext()):
             for rep in range(repeat):
              for l in range(L):
                hsrc = xfull_d if (l == 0 or no_cc) else hf[l - 1]
                b1c = bias_t[:, 0 * L + l:0 * L + l + 1]
                b2c = bias_t[:, 1 * L + l:1 * L + l + 1]
                gac = bias_t[:, 2 * L + l:2 * L + l + 1]
                bec = bias_t[:, 3 * L + l:3 * L + l + 1]
                w1c = w1_t[:, l * F:(l + 1) * F]
                w2c = w2_t[:, l * F:(l + 1) * F]

                ssum = statp.tile([P, T], dt.float32, tag=f"ssum{l}")
                ssq = statp.tile([P, T], dt.float32, tag=f"ssq{l}")

                for t in range(T):
                    g_t = gpool.tile([P, 4 * BCPT, P], dt.float32, tag="G")
                    if no_gather:
                        nc.scalar.copy(g_t[:, 0, :], iota_t[:])
                    for b in range(4 if not no_gather else 0):
                        call = t * 4 + b
                        nc.gpsimd.dma_gather(
                            out_ap=g_t[:, b * BCPT:(b + 1) * BCPT, :],
                            in_ap=hsrc.ap()[b * cfg.BANK_ROWS:
                                            (b + 1) * cfg.BANK_ROWS, :],
                            idxs_ap=gidx_t[:, call * (CAPB // 16):
                                           (call + 1) * (CAPB // 16)],
                            num_idxs=CAPB,
                            num_idxs_reg=CAPB,
                            elem_size=F,
                            queue_num=b % num_swdge_queues,
                        )
                    agg_ps = psA.tile([P, P], dt.float32, tag="agg")
                    nch = 4 * BCPT
                    for ch in range(nch):
                        s_t = spool.tile([P, P], dt.float32, tag="S")
                        nc.vector.tensor_scalar(
                            out=s_t[:], in0=iota_t[:],
                            scalar1=drel_t[:, t * nch + ch:t * nch + ch + 1],
                            scalar2=None, op0=mybir.AluOpType.is_equal)
                        nc.tensor.matmul(agg_ps[:], lhsT=s_t[:],
                                         rhs=g_t[:, 0 if no_gather else ch, :],
                                         start=(ch == 0), stop=(ch == nch - 1))
                    # node-major agg -> SBUF -> transpose to feature-major
                    agg_nm = work.tile([P, P], dt.float32, tag="aggnm")
                    nc.scalar.copy(agg_nm[:], agg_ps[:])
                    aggT_ps = psT.tile([P, P], dt.float32, tag="aggT")
                    nc.tensor.transpose(aggT_ps[:], agg_nm[:], ident_t[:])
                    z1in = work.tile([P, P], dt.float32, tag="z1in")
                    nc.vector.tensor_add(z1in[:], h_loc[:, t * P:(t + 1) * P],
                                         aggT_ps[:])
                    # MLP (feature-major, weights stationary)
                    mp1 = psM.tile([P, P], dt.float32, tag="mp")
                    nc.tensor.matmul(mp1[:], lhsT=w1c, rhs=z1in[:],
                                     start=True, stop=True)
                    z1 = work.tile([P, P], dt.float32, tag="z1")
                    nc.scalar.activation(z1[:], mp1[:],
                                         mybir.ActivationFunctionType.Relu,
                                         bias=b1c)
                    mp2 = psM.tile([P, P], dt.float32, tag="mp")
                    nc.tensor.matmul(mp2[:], lhsT=w2c, rhs=z1[:],
                                     start=True, stop=True)
                    # z_pre overwrites h_loc tile in place
                    nc.scalar.activation(h_loc[:, t * P:(t + 1) * P], mp2[:],
                                         mybir.ActivationFunctionType.Relu,
                                         bias=b2c)
                    # zero phantom (unfilled + pad) seat columns, then stats
                    msk = spool.tile([P, P], dt.float32, tag="S")
                    nc.vector.tensor_scalar(
                        out=msk[:], in0=iota_t[:],
                        scalar1=filled_t[:, t:t + 1], scalar2=None,
                        op0=mybir.AluOpType.is_lt)
                    nc.vector.tensor_tensor(
                        out=h_loc[:, t * P:(t + 1) * P],
                        in0=h_loc[:, t * P:(t + 1) * P], in1=msk[:],
                        op=mybir.AluOpType.mult)
                    seat_ap = h_loc[:, t * P:(t + 1) * P]
                    nc.vector.tensor_reduce(
                        out=ssum[:, t:t + 1], in_=seat_ap,
                        axis=mybir.AxisListType.X, op=mybir.AluOpType.add)
                    sqs = work.tile([P, P], dt.float32, tag="sqs")
                    nc.scalar.activation(sqs[:], seat_ap,
                                         mybir.ActivationFunctionType.Square,
                                         accum_out=ssq[:, t:t + 1])

                # ---- BN stats allreduce ----
                red = work.tile([P, 2], dt.float32, tag="red")
                nc.vector.tensor_reduce(out=red[:, 0:1], in_=ssum[:],
                                        axis=mybir.AxisListType.X,
                                        op=mybir.AluOpType.add)
                nc.vector.tensor_reduce(out=red[:, 1:2], in_=ssq[:],
                                        axis=mybir.AxisListType.X,
                                        op=mybir.AluOpType.add)
                nc.sync.dma_start(st_in[l].ap(), red[:])
                if not no_cc:
                    nc.gpsimd.collective_compute(
                        "AllReduce", mybir.AluOpType.add, replica_groups=groups,
                        ins=[st_in[l].ap().opt()], outs=[st_out[l].ap().opt()])
                stt = work.tile([P, 2], dt.float32, tag="stt")
                nc.sync.dma_start(stt[:], (st_in[l] if no_cc else st_out[l]).ap())
                # mean, var, scale a, shift c
                mean = work.tile([P, 1], dt.float32, tag="mean")
                nc.vector.tensor_scalar(out=mean[:], in0=stt[:, 0:1],
                                        scalar1=inv_n, scalar2=None,
                                        op0=mybir.AluOpType.mult)
                var = work.tile([P, 1], dt.float32, tag="var")
                nc.vector.tensor_scalar(out=var[:], in0=stt[:, 1:2],
                                        scalar1=inv_n, scalar2=None,
                                        op0=mybir.AluOpType.mult)
                msq = work.tile([P, 1], dt.float32, tag="msq")
                nc.vector.tensor_tensor(out=msq[:], in0=mean[:], in1=mean[:],
                                        op=mybir.AluOpType.mult)
                nc.vector.tensor_tensor(out=var[:], in0=var[:], in1=msq[:],
                                        op=mybir.AluOpType.subtract)
                nc.vector.tensor_scalar(out=var[:], in0=var[:],
                                        scalar1=BN_EPS, scalar2=None,
                                        op0=mybir.AluOpType.add)
                sd = work.tile([P, 1], dt.float32, tag="sd")
                nc.scalar.activation(sd[:], var[:],
                                     mybir.ActivationFunctionType.Sqrt)
                inv = work.tile([P, 1], dt.float32, tag="inv")
                nc.vector.reciprocal(inv[:], sd[:])
                a_t = work.tile([P, 1], dt.float32, tag="a")
                nc.vector.tensor_tensor(out=a_t[:], in0=inv[:], in1=gac,
                                        op=mybir.AluOpType.mult)
                c_t = work.tile([P, 1], dt.float32, tag="c")
                nc.vector.tensor_tensor(out=c_t[:], in0=mean[:], in1=a_t[:],
                                        op=mybir.AluOpType.mult)
                nc.vector.tensor_tensor(out=c_t[:], in0=bec, in1=c_t[:],
                                        op=mybir.AluOpType.subtract)

                # ---- normalize in place, transpose, pool, store ----
                pool_ps = psP.tile([P, P], dt.float32, tag="pool")
                for t in range(T):
                    nc.scalar.activation(
                        h_loc[:, t * P:(t + 1) * P],
                        h_loc[:, t * P:(t + 1) * P],
                        mybir.ActivationFunctionType.Identity,
                        bias=c_t[:, 0:1], scale=a_t[:, 0:1])
                    zT_ps = psT.tile([P, P], dt.float32, tag="aggT")
                    nc.tensor.transpose(zT_ps[:], h_loc[:, t * P:(t + 1) * P],
                                        ident_t[:])
                    znm = work.tile([P, P], dt.float32, tag="znm")
                    nc.scalar.copy(znm[:], zT_ps[:])
                    if l < L - 1:
                        nc.sync.dma_start(zz[l].ap()[t * P:(t + 1) * P, :],
                                          znm[:])
                    sb_t = spool.tile([P, P], dt.float32, tag="S")
                    nc.vector.tensor_scalar(
                        out=sb_t[:], in0=iota_t[:],
                        scalar1=brel_t[:, t:t + 1], scalar2=None,
                        op0=mybir.AluOpType.is_equal)
                    nc.tensor.matmul(pool_ps[:], lhsT=sb_t[:], rhs=znm[:],
                                     start=(t == 0), stop=(t == T - 1),
                                     skip_group_check=True)
                poolsb = work.tile([P, P], dt.float32, tag="poolsb")
                nc.scalar.copy(poolsb[:], pool_ps[:])
                nc.sync.dma_start(pools_d.ap()[:, l * F:(l + 1) * F],
                                  poolsb[:])

                if l < L - 1 and not no_cc:
                    nc.gpsimd.collective_compute(
                        "AllGather", mybir.AluOpType.bypass,
                        replica_groups=groups,
                        ins=[zz[l].ap().opt()], outs=[hf[l].ap().opt()])

    nc.compile()
    return nc


def make_in_maps(per_core, x_full, inputs):
    W1 = np.asarray(inputs['W1'], np.float32)
    W2 = np.asarray(inputs['W2'], np.float32)
    b1 = np.asarray(inputs['b1'], np.float32)
    b2 = np.asarray(inputs['b2'], np.float32)
    gamma = np.asarray(inputs['gamma'], np.float32)
    beta = np.asarray(inputs['beta'], np.float32)
    w1_h = np.ascontiguousarray(np.concatenate([W1[i] for i in range(L)], 1))
    w2_h = np.ascontiguousarray(np.concatenate([W2[i] for i in range(L)], 1))
    bias_h = np.ascontiguousarray(
        np.concatenate([b1.T, b2.T, gamma.T, beta.T], 1))
    iota_h = np.tile(np.arange(P, dtype=np.float32), (P, 1))
    ident_h = np.eye(P, dtype=np.float32)
    in_maps = []
    for c in range(NCORES):
        pc = per_core[c]
        in_maps.append({
            "xT": pc["xT"], "xnm": pc["xnm"], "gidx": pc["gidx"],
            "dstrel": pc["dstrel"], "brel": pc["batch_rel"],
            "filled": pc["filled"],
            "w1": w1_h, "w2": w2_h, "bias": bias_h,
            "iotat": iota_h, "ident": ident_h,
        })
    return in_maps


def build_null_kernel(cfg, BCPT):
    """Same I/O signature as the real kernel, trivial body. Used to measure
    the non-compute overhead (upload/dispatch) of a run for timing deltas."""
    CAPB = BCPT * 128
    T = cfg.T
    dt = mybir.dt
    nc = bacc.Bacc("TRN2", target_bir_lowering=False, debug=False,
                   enable_asserts=True, num_devices=NCORES,
                   num_swdge_queues=4)
    xT_d = nc.dram_tensor("xT", [P, cfg.COLS], dt.float32,
                          kind="ExternalInput")
    nc.dram_tensor("xnm", [cfg.COLS, F], dt.float32, kind="ExternalInput")
    nc.dram_tensor("gidx", [P, T * 4 * (CAPB // 16)], dt.int16,
                   kind="ExternalInput")
    nc.dram_tensor("dstrel", [P, T * 4 * BCPT], dt.float32,
                   kind="ExternalInput")
    nc.dram_tensor("brel", [P, T], dt.float32, kind="ExternalInput")
    nc.dram_tensor("filled", [P, T], dt.float32, kind="ExternalInput")
    nc.dram_tensor("w1", [P, L * F], dt.float32, kind="ExternalInput")
    nc.dram_tensor("w2", [P, L * F], dt.float32, kind="ExternalInput")
    nc.dram_tensor("bias", [P, 4 * L], dt.float32, kind="ExternalInput")
    nc.dram_tensor("iotat", [P, P], dt.float32, kind="ExternalInput")
    nc.dram_tensor("ident", [P, P], dt.float32, kind="ExternalInput")
    pools_d = nc.dram_tensor("pools", [P, L * F], dt.float32,
                             kind="ExternalOutput")
    with tile.TileContext(nc) as tc:
        with tc.tile_pool(name="sb", bufs=1) as sb:
            t0 = sb.tile([P, L * F], dt.float32)
            nc.sync.dma_start(t0[:], xT_d.ap()[:, 0:L * F])
            nc.sync.dma_start(pools_d.ap(), t0[:])
    nc.compile()
    return nc


# ---------------- entry point ----------------
def kernel(x, edge_index, batch, W1, b1, W2, b2, gamma, beta):
    cfg = FULL
    x = np.asarray(x, np.float32)
    edge_index = np.asarray(edge_index, np.int32)
    batch = np.asarray(batch, np.int32)
    W1 = np.asarray(W1, np.float32)
    b1 = np.asarray(b1, np.float32)
    W2 = np.asarray(W2, np.float32)
    b2 = np.asarray(b2, np.float32)
    gamma = np.asarray(gamma, np.float32)
    beta = np.asarray(beta, np.float32)

    x_full, per_core, meta = preprocess(cfg, x, edge_index, batch)
    nc = build_kernel(cfg, meta['BCPT'])
    in_maps = make_in_maps(per_core, x_full, dict(
        W1=W1, W2=W2, b1=b1, b2=b2, gamma=gamma, beta=beta))

    import time as _time
    last_exc = None
    for attempt in range(3):
        try:
            res = bass_utils.run_bass_kernel_spmd(
                nc, in_maps, core_ids=list(range(NCORES)))
            break
        except Exception as e:       # transient device wedge -> retry
            last_exc = e
            _time.sleep(20)
    else:
        raise last_exc

    out = np.zeros((NUM_GRAPHS, L * F), np.float32)
    for c in range(NCORES):
        pool_c = res.results[c]["pools"]          # [128, L*F]
        g0, sp = meta['g_bases'][c], meta['spans'][c]
        out[g0:g0 + sp] += pool_c[:sp]
    return out


if __name__ == "__main__":
    import reference
    inputs = reference.setup_inputs()
    inputs = {k: np.asarray(v) for k, v in inputs.items()}
    got = kernel(**inputs)
    print("kernel output shape:", got.shape)



# revision 25
# speedup vs baseline: 47.8842x; 1.0600x over previous
"""GIN encoder (3x GINConv+BN + per-layer global_add_pool) on 8 Trainium2 cores.

Strategy:
  - Nodes sharded round 8 cores (12500 each). Each core's nodes are permuted
    into 102 "tiles" of 125 seats (+3 pad) chosen to balance gather traffic
    across 4 source banks (degree-sorted round-robin + local repair).
  - Edges sharded by dst owner; per (dst-tile, src-bank) edge slots padded to
    a fixed cap -> fully uniform SPMD program.
  - Aggregation h[src] gather via dma_gather (4 SWDGE queues) from a
    [104448, 128] node-feature table (AllGather of per-core node-major
    slices for layer 0, AllGather outputs for layers 1, 2), then
    segment-sum as one-hot matmuls accumulating feature-major in PSUM.
  - MLP feature-major on PE (weights stationary), BN stats via [128,2]
    AllReduce, pool via one-hot matmul; host sums per-core partial pools.
  - All per-tile pipelines run inside For_i hardware loops with
    register-indexed (DynSlice) APs: ~1.1k static instructions vs ~24k
    fully unrolled. This runtime charges ~130us per static instruction
    per execution, so static code size dominates device time; hardware
    loop re-execution is nearly free (measured ~10ns/instruction).
"""
import sys
sys.path.insert(0, '/opt/trn_rl_repo')

import numpy as np

import concourse.bass as bass
import concourse.tile as tile
from concourse import bacc, mybir, library_config
from concourse import bass_utils

# ---------------- problem config (hardcoded from spec) ----------------
NCORES = 8
N = 100000
F = 128
E = 1600000
L = 3
NUM_GRAPHS = 512
BN_EPS = 1e-5
P = 128


class Cfg:
    def __init__(self, N, E, num_graphs, T, seats, capb):
        self.N = N
        self.E = E
        self.num_graphs = num_graphs
        self.NPC = N // NCORES            # nodes per core
        self.T = T                        # dst tiles per core
        self.SEATS = seats                # real seats per tile (<=128)
        self.CAPB = capb                  # slots per (tile, bank); %128==0
        self.BCPT = capb // 128           # chunks per (tile, bank)
        self.COLS = T * P                 # padded node columns per core
        self.HF_ROWS = NCORES * self.COLS
        self.NBANK = 4
        self.BANK_ROWS = self.HF_ROWS // 4
        assert self.BANK_ROWS < 32767
        assert self.T * self.SEATS >= self.NPC
        assert self.CAPB <= 1024 and self.CAPB % 128 == 0


FULL = Cfg(N, E, NUM_GRAPHS, T=102, seats=125, capb=512)


# ---------------- host-side preprocessing ----------------
def preprocess(cfg, x, edge_index, batch):
    """Returns per-core input dicts + metadata for output assembly."""
    NPC, T, SEATS = cfg.NPC, cfg.T, cfg.SEATS
    src = edge_index[0].astype(np.int64)
    dst = edge_index[1].astype(np.int64)
    owner = dst // NPC
    src_bank = src // (2 * NPC)          # = owner(src)//2, permutation-invariant

    # --- per-core tile assignment balancing per-bank degree ---
    tile_of_g = np.empty(cfg.N, np.int64)     # local tile of each node
    seat_of_g = np.empty(cfg.N, np.int64)     # seat within tile
    capb_used = cfg.CAPB
    for c in range(NCORES):
        m = owner == c
        d_loc = dst[m] - c * NPC
        counts = np.zeros((NPC, 4), np.int64)
        np.add.at(counts, (d_loc, src_bank[m]), 1)
        deg = counts.sum(1)
        order = np.argsort(-deg, kind='stable')

        # fast path: degree-sorted round-robin + local repair of cap
        # violations; falls back to the exact greedy below if repair fails
        ok_all = False
        tile_of = np.empty(NPC, np.int64)
        tile_of[order] = np.arange(NPC) % T
        loads = np.zeros((T, 4), np.int64)
        for b in range(4):
            loads[:, b] = np.bincount(tile_of, weights=counts[:, b],
                                      minlength=T).astype(np.int64)
        nseat = np.bincount(tile_of, minlength=T)
        if nseat.max() <= SEATS:
            for _repair in range(200):
                over = np.argwhere(loads > cfg.CAPB)
                if len(over) == 0:
                    ok_all = True
                    break
                t_bad, b_bad = over[0]
                cand = np.where(tile_of == t_bad)[0]
                cand = cand[np.argsort(-counts[cand, b_bad])]
                moved = False
                for d in cand:
                    room = ((loads + counts[d] <= cfg.CAPB).all(1)
                            & (nseat < SEATS))
                    room[t_bad] = False
                    tgt = np.argwhere(room)
                    if len(tgt):
                        t_new = int(tgt[0][0])
                        loads[t_bad] -= counts[d]
                        loads[t_new] += counts[d]
                        nseat[t_bad] -= 1
                        nseat[t_new] += 1
                        tile_of[d] = t_new
                        moved = True
                        break
                if not moved:
                    break
        if ok_all:
            seat_of = np.empty(NPC, np.int64)
            order_t = np.argsort(tile_of, kind='stable')
            tt = tile_of[order_t]
            starts = np.searchsorted(tt, np.arange(T))
            seat_of[order_t] = np.arange(NPC) - np.repeat(
                starts, np.bincount(tt, minlength=T))
            tile_of_g[c * NPC:(c + 1) * NPC] = tile_of
            seat_of_g[c * NPC:(c + 1) * NPC] = seat_of
            continue

        for cap_try in (cfg.CAPB, cfg.CAPB + 128, cfg.CAPB + 256, 1024):
            rem = np.full((T, 4), cap_try, np.int64)
            seats = np.full(T, SEATS, np.int64)
            tile_of = np.full(NPC, -1, np.int64)
            seat_of = np.full(NPC, -1, np.int64)
            ok_all = True
            for d in order:
                v = counts[d]
                feas = (seats > 0) & (rem >= v).all(1)
                if not feas.any():
                    ok_all = False
                    break
                slack = (rem - v).min(1).astype(np.float64)
                slack[~feas] = -1e18
                t = int(np.argmax(slack))
                tile_of[d] = t
                seat_of[d] = SEATS - seats[t]
                rem[t] -= v
                seats[t] -= 1
            if ok_all:
                capb_used = max(capb_used, cap_try)
                break
        assert ok_all, "tile assignment failed even at cap 1024"
        tile_of_g[c * NPC:(c + 1) * NPC] = tile_of
        seat_of_g[c * NPC:(c + 1) * NPC] = seat_of

    CAPB = capb_used
    BCPT = CAPB // 128
    perm_pos = tile_of_g * P + seat_of_g                     # within-core col
    perm_row = (np.arange(cfg.N) // NPC) * cfg.COLS + perm_pos  # global row

    # --- x tables ---
    xf = np.asarray(x, np.float32)
    x_full = np.zeros((cfg.HF_ROWS, F), np.float32)
    x_full[perm_row] = xf
    g_bases, spans = [], []
    per_core = []
    for c in range(NCORES):
        lo, hi = c * NPC, (c + 1) * NPC
        xnm = np.zeros((cfg.COLS, F), np.float32)            # node-major
        xnm[perm_pos[lo:hi]] = xf[lo:hi]
        xlT = np.ascontiguousarray(xnm.T)                    # [128, COLS]

        g_base = int(batch[lo])
        span = int(batch[hi - 1]) - g_base + 1
        assert span <= P, f"graph span {span} exceeds 128"
        g_bases.append(g_base)
        spans.append(span)
        br = np.full(cfg.COLS, -1.0, np.float32)
        br[perm_pos[lo:hi]] = (batch[lo:hi] - g_base).astype(np.float32)
        batch_rel = np.ascontiguousarray(br.reshape(T, P).T)  # [128, T]
        filled_row = np.bincount(tile_of_g[lo:hi], minlength=T).astype(np.float32)
        filled_tbl = np.tile(filled_row, (P, 1))              # [128, T]

        # --- edge slot tables ---
        m = owner == c
        e_src = src[m]
        e_dst = dst[m] - c * NPC
        key = tile_of_g[c * NPC + e_dst] * 4 + src_bank[m]
        order_e = np.argsort(key, kind='stable')
        key_s = key[order_e]
        cnt = np.bincount(key_s, minlength=T * 4)
        assert cnt.max() <= CAPB, f"(tile,bank) count {cnt.max()} > cap {CAPB}"
        cstart = np.zeros(T * 4, np.int64)
        cstart[1:] = np.cumsum(cnt)[:-1]
        within = np.arange(len(key_s)) - np.repeat(cstart, cnt)
        slot = key_s * CAPB + within
        gidx_flat = np.zeros(T * 4 * CAPB, np.int64)
        drel_flat = np.full(T * 4 * CAPB, -1.0, np.float32)
        gidx_flat[slot] = perm_row[e_src[order_e]] % cfg.BANK_ROWS
        drel_flat[slot] = seat_of_g[c * NPC + e_dst[order_e]]
        # idx wrap: per call (t,b): CAPB values -> [16, CAPB//16], tile x8
        w = gidx_flat.reshape(T * 4, CAPB // 16, 16).transpose(2, 0, 1)
        w = w.reshape(16, T * 4 * (CAPB // 16))
        gidx_h = np.tile(w, (8, 1)).astype(np.int16)         # [128, cols]
        drel_h = np.ascontiguousarray(
            drel_flat.reshape(T * 4 * BCPT, P).T)             # [128, chunks]

        per_core.append(dict(xT=xlT, xnm=xnm, batch_rel=batch_rel,
                             gidx=gidx_h, dstrel=drel_h, filled=filled_tbl))

    meta = dict(CAPB=CAPB, BCPT=BCPT, g_bases=g_bases, spans=spans)
    return x_full, per_core, meta


# ---------------- device kernel builder (v2: hardware loops) ----------------
def build_kernel_v2(cfg, BCPT, num_swdge_queues=4, repeat=1, no_cc=False,
                    fat_onehot=True, t_frac=1.0):
    """Same algorithm as v1 but the per-tile pipelines run inside For_i
    hardware loops with register-indexed APs. Static instruction count
    ~220 vs ~15.6k; this runtime charges ~127us per STATIC instruction,
    so this is the dominant win."""
    from concourse.bass import ds
    CAPB = BCPT * 128
    T = cfg.T
    nch = 4 * BCPT
    C16 = CAPB // 16
    dt = mybir.dt
    nc = bacc.Bacc("TRN2", target_bir_lowering=False, debug=False,
                   enable_asserts=True, num_devices=NCORES,
                   num_swdge_queues=num_swdge_queues)

    # inputs (v1 signature minus the replicated xfull table: the layer-0
    # gather table is AllGathered on device from the per-core xT slices)
    xT_d = nc.dram_tensor("xT", [P, cfg.COLS], dt.float32, kind="ExternalInput")
    gidx_d = nc.dram_tensor("gidx", [P, T * 4 * C16], dt.int16,
                            kind="ExternalInput")
    dstrel_d = nc.dram_tensor("dstrel", [P, T * nch], dt.float32,
                              kind="ExternalInput")
    xnm_in = nc.dram_tensor("xnm", [cfg.COLS, F], dt.float32,
                            kind="ExternalInput")
    xnm_d = nc.dram_tensor("xnmi", [cfg.COLS, F], dt.float32, kind="Internal")
    xg_d = nc.dram_tensor("xg", [cfg.HF_ROWS, F], dt.float32,
                          kind="Internal", addr_space="Shared")
    brel_d = nc.dram_tensor("brel", [P, T], dt.float32, kind="ExternalInput")
    filled_d = nc.dram_tensor("filled", [P, T], dt.float32, kind="ExternalInput")
    w1_d = nc.dram_tensor("w1", [P, L * F], dt.float32, kind="ExternalInput")
    w2_d = nc.dram_tensor("w2", [P, L * F], dt.float32, kind="ExternalInput")
    bias_d = nc.dram_tensor("bias", [P, 4 * L], dt.float32, kind="ExternalInput")
    iota_d = nc.dram_tensor("iotat", [P, P], dt.float32, kind="ExternalInput")
    ident_d = nc.dram_tensor("ident", [P, P], dt.float32, kind="ExternalInput")
    pools_d = nc.dram_tensor("pools", [P, L * F], dt.float32,
                             kind="ExternalOutput")

    zz = [nc.dram_tensor(f"zz{l}", [cfg.COLS, F], dt.float32, kind="Internal")
          for l in range(L - 1)]
    hf = [nc.dram_tensor(f"hf{l}", [cfg.HF_ROWS, F], dt.float32,
                         kind="Internal", addr_space="Shared")
          for l in range(L - 1)]
    st_in = [nc.dram_tensor(f"stin{l}", [P, 2], dt.float32, kind="Internal")
             for l in range(L)]
    st_out = [nc.dram_tensor(f"stout{l}", [P, 2], dt.float32, kind="Internal",
                             addr_space="Shared") for l in range(L)]

    inv_n = 1.0 / cfg.N
    groups = [list(range(NCORES))]

    with tile.TileContext(nc) as tc:
        with tc.tile_pool(name="big", bufs=1) as big, \
             tc.tile_pool(name="gpool", bufs=2) as gpool, \
             tc.tile_pool(name="spool", bufs=2) as spool, \
             tc.tile_pool(name="work", bufs=2) as work, \
             tc.tile_pool(name="stat", bufs=1) as statp, \
             tc.tile_pool(name="psA", bufs=2, space="PSUM") as psA, \
             tc.tile_pool(name="psT", bufs=2, space="PSUM") as psT, \
             tc.tile_pool(name="psM", bufs=2, space="PSUM") as psM, \
             tc.tile_pool(name="psP", bufs=2, space="PSUM") as psP:

            nc.gpsimd.load_library(library_config.mlp)

            h_loc = big.tile([P, cfg.COLS], dt.float32)
            nc.sync.dma_start(h_loc[:], xT_d.ap())
            gidx_t = big.tile([P, T * 4 * C16], dt.int16)
            nc.sync.dma_start(gidx_t[:], gidx_d.ap())
            drel_t = big.tile([P, T * nch], dt.float32)
            nc.sync.dma_start(drel_t[:], dstrel_d.ap())
            brel_t = big.tile([P, T], dt.float32)
            nc.sync.dma_start(brel_t[:], brel_d.ap())
            filled_t = big.tile([P, T], dt.float32)
            nc.sync.dma_start(filled_t[:], filled_d.ap())
            w1_t = big.tile([P, L * F], dt.float32)
            nc.sync.dma_start(w1_t[:], w1_d.ap())
            w2_t = big.tile([P, L * F], dt.float32)
            nc.sync.dma_start(w2_t[:], w2_d.ap())
            bias_t = big.tile([P, 4 * L], dt.float32)
            nc.sync.dma_start(bias_t[:], bias_d.ap())
            iota_t = big.tile([P, P], dt.float32)
            nc.sync.dma_start(iota_t[:], iota_d.ap())
            ident_t = big.tile([P, P], dt.float32)
            nc.sync.dma_start(ident_t[:], ident_d.ap())

            # layer-0 gather table: AllGather the host-provided node-major x
            # (collective ins must be internal DRAM -> one local copy first)
            nc.sync.dma_start(xnm_d.ap(), xnm_in.ap())
            if not no_cc:
                nc.gpsimd.collective_compute(
                    "AllGather", mybir.AluOpType.bypass,
                    replica_groups=groups,
                    ins=[xnm_d.ap().opt()], outs=[xg_d.ap().opt()])

            for rep in range(repeat):
             for l in range(L):
                hsrc = xg_d if (l == 0 or no_cc) else hf[l - 1]
                b1c = bias_t[:, 0 * L + l:0 * L + l + 1]
                b2c = bias_t[:, 1 * L + l:1 * L + l + 1]
                gac = bias_t[:, 2 * L + l:2 * L + l + 1]
                bec = bias_t[:, 3 * L + l:3 * L + l + 1]
                w1c = w1_t[:, l * F:(l + 1) * F]
                w2c = w2_t[:, l * F:(l + 1) * F]

                acc = statp.tile([P, 2], dt.float32, tag=f"acc{l}{rep}")
                nc.vector.memset(acc[:], 0.0)

                # ---- phase A: aggregate + MLP, HW loop over dst tiles ----
                with tc.For_i(0, max(1, int(T * t_frac)), 1) as t:
                    g_t = gpool.tile([P, nch, P], dt.float32, tag="G")
                    for b in range(4):
                        nc.gpsimd.dma_gather(
                            out_ap=g_t[:, b * BCPT:(b + 1) * BCPT, :],
                            in_ap=hsrc.ap()[b * cfg.BANK_ROWS:
                                            (b + 1) * cfg.BANK_ROWS, :],
                            idxs_ap=gidx_t[:, ds(t * (4 * C16) + b * C16, C16)],
                            num_idxs=CAPB,
                            num_idxs_reg=CAPB,
                            elem_size=F,
                            queue_num=b % num_swdge_queues,
                        )
                    # one-hot seat matrices for all chunks
                    if fat_onehot:
                        s_all = spool.tile([P, nch, P], dt.float32, tag="S")
                        nc.vector.tensor_tensor(
                            out=s_all[:],
                            in0=iota_t[:].unsqueeze(1)
                                .broadcast_to([P, nch, P]),
                            in1=drel_t[:, ds(t * nch, nch)].unsqueeze(2)
                                .broadcast_to([P, nch, P]),
                            op=mybir.AluOpType.is_equal)
                    else:
                        s_all = spool.tile([P, nch, P], dt.float32, tag="S")
                        for ch in range(nch):
                            nc.vector.tensor_scalar(
                                out=s_all[:, ch, :], in0=iota_t[:],
                                scalar1=drel_t[:, ds(t * nch + ch, 1)],
                                scalar2=None, op0=mybir.AluOpType.is_equal)
                    # feature-major agg: agg[f, seat] += g[e,f]^T @ onehot[e,seat]
                    agg_ps = psA.tile([P, P], dt.float32, tag="agg")
                    for ch in range(nch):
                        nc.tensor.matmul(agg_ps[:], lhsT=g_t[:, ch, :],
                                         rhs=s_all[:, ch, :],
                                         start=(ch == 0), stop=(ch == nch - 1))
                    z1in = work.tile([P, P], dt.float32, tag="z1in")
                    nc.vector.tensor_add(z1in[:], h_loc[:, ds(t * P, P)],
                                         agg_ps[:])
                    mp1 = psM.tile([P, P], dt.float32, tag="mp")
                    nc.tensor.matmul(mp1[:], lhsT=w1c, rhs=z1in[:],
                                     start=True, stop=True)
                    z1 = work.tile([P, P], dt.float32, tag="z1")
                    nc.scalar.activation(z1[:], mp1[:],
                                         mybir.ActivationFunctionType.Relu,
                                         bias=b1c)
                    mp2 = psM.tile([P, P], dt.float32, tag="mp")
                    nc.tensor.matmul(mp2[:], lhsT=w2c, rhs=z1[:],
                                     start=True, stop=True)
                    nc.scalar.activation(h_loc[:, ds(t * P, P)], mp2[:],
                                         mybir.ActivationFunctionType.Relu,
                                         bias=b2c)
                    msk = work.tile([P, P], dt.float32, tag="msk")
                    nc.vector.tensor_scalar(
                        out=msk[:], in0=iota_t[:],
                        scalar1=filled_t[:, ds(t, 1)], scalar2=None,
                        op0=mybir.AluOpType.is_lt)
                    nc.vector.tensor_tensor(
                        out=h_loc[:, ds(t * P, P)],
                        in0=h_loc[:, ds(t * P, P)], in1=msk[:],
                        op=mybir.AluOpType.mult)
                    s12 = work.tile([P, 2], dt.float32, tag="s12")
                    nc.vector.tensor_reduce(
                        out=s12[:, 0:1], in_=h_loc[:, ds(t * P, P)],
                        axis=mybir.AxisListType.X, op=mybir.AluOpType.add)
                    sqs = work.tile([P, P], dt.float32, tag="sqs")
                    nc.scalar.activation(sqs[:], h_loc[:, ds(t * P, P)],
                                         mybir.ActivationFunctionType.Square,
                                         accum_out=s12[:, 1:2])
                    nc.vector.tensor_tensor(out=acc[:], in0=acc[:],
                                            in1=s12[:],
                                            op=mybir.AluOpType.add)

                # ---- BN stats allreduce ----
                nc.sync.dma_start(st_in[l].ap(), acc[:])
                if not no_cc:
                    nc.gpsimd.collective_compute(
                        "AllReduce", mybir.AluOpType.add, replica_groups=groups,
                        ins=[st_in[l].ap().opt()], outs=[st_out[l].ap().opt()])
                stt = work.tile([P, 2], dt.float32, tag="stt")
                nc.sync.dma_start(stt[:], (st_in[l] if no_cc else st_out[l]).ap())
                mean = work.tile([P, 1], dt.float32, tag="mean")
                nc.vector.tensor_scalar(out=mean[:], in0=stt[:, 0:1],
                                        scalar1=inv_n, scalar2=None,
                                        op0=mybir.AluOpType.mult)
                msq = work.tile([P, 1], dt.float32, tag="msq")
                nc.vector.tensor_tensor(out=msq[:], in0=mean[:], in1=mean[:],
                                        op=mybir.AluOpType.mult)
                var = work.tile([P, 1], dt.float32, tag="var")
                nc.vector.tensor_scalar(out=var[:], in0=stt[:, 1:2],
                                        scalar1=inv_n, scalar2=BN_EPS,
                                        op0=mybir.AluOpType.mult,
                                        op1=mybir.AluOpType.add)
                nc.vector.tensor_tensor(out=var[:], in0=var[:], in1=msq[:],
                                        op=mybir.AluOpType.subtract)
                sd = work.tile([P, 1], dt.float32, tag="sd")
                nc.scalar.activation(sd[:], var[:],
                                     mybir.ActivationFunctionType.Sqrt)
                inv = work.tile([P, 1], dt.float32, tag="inv")
                nc.vector.reciprocal(inv[:], sd[:])
                a_t = work.tile([P, 1], dt.float32, tag="a")
                nc.vector.tensor_tensor(out=a_t[:], in0=inv[:], in1=gac,
                                        op=mybir.AluOpType.mult)
                c_t = work.tile([P, 1], dt.float32, tag="c")
                nc.vector.tensor_tensor(out=c_t[:], in0=mean[:], in1=a_t[:],
                                        op=mybir.AluOpType.mult)
                nc.vector.tensor_tensor(out=c_t[:], in0=bec, in1=c_t[:],
                                        op=mybir.AluOpType.subtract)

                # ---- phase C: normalize, transpose, pool (HW loop) ----
                pool_sb = statp.tile([P, P], dt.float32, tag=f"pool{l}{rep}")
                nc.vector.memset(pool_sb[:], 0.0)
                with tc.For_i(0, max(1, int(T * t_frac)), 1) as t:
                    # walrus matmul lhsT needs a static offset: stage the
                    # normalized tile in a fixed SBUF buffer for the transpose
                    znf = work.tile([P, P], dt.float32, tag="znf")
                    if l < L - 1:
                        nc.scalar.activation(
                            h_loc[:, ds(t * P, P)], h_loc[:, ds(t * P, P)],
                            mybir.ActivationFunctionType.Identity,
                            bias=c_t[:, 0:1], scale=a_t[:, 0:1])
                        nc.scalar.copy(znf[:], h_loc[:, ds(t * P, P)])
                    else:
                        nc.scalar.activation(
                            znf[:], h_loc[:, ds(t * P, P)],
                            mybir.ActivationFunctionType.Identity,
                            bias=c_t[:, 0:1], scale=a_t[:, 0:1])
                    zT_ps = psT.tile([P, P], dt.float32, tag="zT")
                    nc.tensor.transpose(zT_ps[:], znf[:], ident_t[:])
                    znm = work.tile([P, P], dt.float32, tag="znm")
                    nc.scalar.copy(znm[:], zT_ps[:])
                    if l < L - 1:
                        nc.sync.dma_start(zz[l].ap()[ds(t * P, P), :], znm[:])
                    sb_t = spool.tile([P, P], dt.float32, tag="SB")
                    nc.vector.tensor_scalar(
                        out=sb_t[:], in0=iota_t[:],
                        scalar1=brel_t[:, ds(t, 1)], scalar2=None,
                        op0=mybir.AluOpType.is_equal)
                    pp = psP.tile([P, P], dt.float32, tag="pp")
                    nc.tensor.matmul(pp[:], lhsT=sb_t[:], rhs=znm[:],
                                     start=True, stop=True)
                    nc.vector.tensor_add(pool_sb[:], pool_sb[:], pp[:])
                nc.sync.dma_start(pools_d.ap()[:, l * F:(l + 1) * F],
                                  pool_sb[:])

                if l < L - 1 and not no_cc:
                    nc.gpsimd.collective_compute(
                        "AllGather", mybir.AluOpType.bypass,
                        replica_groups=groups,
                        ins=[zz[l].ap().opt()], outs=[hf[l].ap().opt()])

    nc.compile()
    return nc


def build_kernel(cfg, BCPT, **kw):
    return build_kernel_v2(cfg, BCPT, **kw)


# ---------------- v1 (fully unrolled; kept for reference/fallback) ----------
def build_kernel_v1(cfg, BCPT, num_swdge_queues=4, repeat=1, loop_n=1, no_cc=False, no_gather=False):
    CAPB = BCPT * 128
    T = cfg.T
    dt = mybir.dt
    nc = bacc.Bacc("TRN2", target_bir_lowering=False, debug=False,
                   enable_asserts=True, num_devices=NCORES,
                   num_swdge_queues=num_swdge_queues)

    # inputs
    xT_d = nc.dram_tensor("xT", [P, cfg.COLS], dt.float32, kind="ExternalInput")
    xfull_d = nc.dram_tensor("xfull", [cfg.HF_ROWS, F], dt.float32,
                             kind="ExternalInput")
    gidx_d = nc.dram_tensor("gidx", [P, T * 4 * (CAPB // 16)], dt.int16,
                            kind="ExternalInput")
    dstrel_d = nc.dram_tensor("dstrel", [P, T * 4 * BCPT], dt.float32,
                              kind="ExternalInput")
    brel_d = nc.dram_tensor("brel", [P, T], dt.float32, kind="ExternalInput")
    filled_d = nc.dram_tensor("filled", [P, T], dt.float32, kind="ExternalInput")
    w1_d = nc.dram_tensor("w1", [P, L * F], dt.float32, kind="ExternalInput")
    w2_d = nc.dram_tensor("w2", [P, L * F], dt.float32, kind="ExternalInput")
    bias_d = nc.dram_tensor("bias", [P, 4 * L], dt.float32, kind="ExternalInput")
    iota_d = nc.dram_tensor("iotat", [P, P], dt.float32, kind="ExternalInput")
    ident_d = nc.dram_tensor("ident", [P, P], dt.float32, kind="ExternalInput")
    pools_d = nc.dram_tensor("pools", [P, L * F], dt.float32,
                             kind="ExternalOutput")

    # internal DRAM
    zz = [nc.dram_tensor(f"zz{l}", [cfg.COLS, F], dt.float32, kind="Internal")
          for l in range(L - 1)]
    hf = [nc.dram_tensor(f"hf{l}", [cfg.HF_ROWS, F], dt.float32,
                         kind="Internal", addr_space="Shared")
          for l in range(L - 1)]
    st_in = [nc.dram_tensor(f"stin{l}", [P, 2], dt.float32, kind="Internal")
             for l in range(L)]
    st_out = [nc.dram_tensor(f"stout{l}", [P, 2], dt.float32, kind="Internal",
                             addr_space="Shared") for l in range(L)]

    inv_n = 1.0 / cfg.N
    groups = [list(range(NCORES))]

    with tile.TileContext(nc) as tc:
        with tc.tile_pool(name="big", bufs=1) as big, \
             tc.tile_pool(name="gpool", bufs=3) as gpool, \
             tc.tile_pool(name="spool", bufs=8) as spool, \
             tc.tile_pool(name="work", bufs=4) as work, \
             tc.tile_pool(name="stat", bufs=1) as statp, \
             tc.tile_pool(name="psA", bufs=2, space="PSUM") as psA, \
             tc.tile_pool(name="psT", bufs=2, space="PSUM") as psT, \
             tc.tile_pool(name="psM", bufs=2, space="PSUM") as psM, \
             tc.tile_pool(name="psP", bufs=1, space="PSUM") as psP:

            nc.gpsimd.load_library(library_config.mlp)

            h_loc = big.tile([P, cfg.COLS], dt.float32)       # feature-major h
            nc.sync.dma_start(h_loc[:], xT_d.ap())
            gidx_t = big.tile([P, T * 4 * (CAPB // 16)], dt.int16)
            nc.sync.dma_start(gidx_t[:], gidx_d.ap())
            drel_t = big.tile([P, T * 4 * BCPT], dt.float32)
            nc.sync.dma_start(drel_t[:], dstrel_d.ap())
            brel_t = big.tile([P, T], dt.float32)
            nc.sync.dma_start(brel_t[:], brel_d.ap())
            filled_t = big.tile([P, T], dt.float32)
            nc.sync.dma_start(filled_t[:], filled_d.ap())
            w1_t = big.tile([P, L * F], dt.float32)
            nc.sync.dma_start(w1_t[:], w1_d.ap())
            w2_t = big.tile([P, L * F], dt.float32)
            nc.sync.dma_start(w2_t[:], w2_d.ap())
            bias_t = big.tile([P, 4 * L], dt.float32)
            nc.sync.dma_start(bias_t[:], bias_d.ap())
            iota_t = big.tile([P, P], dt.float32)
            nc.sync.dma_start(iota_t[:], iota_d.ap())
            ident_t = big.tile([P, P], dt.float32)
            nc.sync.dma_start(ident_t[:], ident_d.ap())

            from contextlib import nullcontext
            with (tc.For_i(0, loop_n, 1) if loop_n > 1 else nullcontext()):
             for rep in range(repeat):
              for l in range(L):
                hsrc = xfull_d if (l == 0 or no_cc) else hf[l - 1]
                b1c = bias_t[:, 0 * L + l:0 * L + l + 1]
                b2c = bias_t[:, 1 * L + l:1 * L + l + 1]
                gac = bias_t[:, 2 * L + l:2 * L + l + 1]
                bec = bias_t[:, 3 * L + l:3 * L + l + 1]
                w1c = w1_t[:, l * F:(l + 1) * F]
                w2c = w2_t[:, l * F:(l + 1) * F]

                ssum = statp.tile([P, T], dt.float32, tag=f"ssum{l}")
                ssq = statp.tile([P, T], dt.float32, tag=f"ssq{l}")

                for t in range(T):
                    g_t = gpool.tile([P, 4 * BCPT, P], dt.float32, tag="G")
                    if no_gather:
                        nc.scalar.copy(g_t[:, 0, :], iota_t[:])
                    for b in range(4 if not no_gather else 0):
                        call = t * 4 + b
                        nc.gpsimd.dma_gather(
                            out_ap=g_t[:, b * BCPT:(b + 1) * BCPT, :],
                            in_ap=hsrc.ap()[b * cfg.BANK_ROWS:
                                            (b + 1) * cfg.BANK_ROWS, :],
                            idxs_ap=gidx_t[:, call * (CAPB // 16):
                                           (call + 1) * (CAPB // 16)],
                            num_idxs=CAPB,
                            num_idxs_reg=CAPB,
                            elem_size=F,
                            queue_num=b % num_swdge_queues,
                        )
                    agg_ps = psA.tile([P, P], dt.float32, tag="agg")
                    nch = 4 * BCPT
                    for ch in range(nch):
                        s_t = spool.tile([P, P], dt.float32, tag="S")
                        nc.vector.tensor_scalar(
                            out=s_t[:], in0=iota_t[:],
                            scalar1=drel_t[:, t * nch + ch:t * nch + ch + 1],
                            scalar2=None, op0=mybir.AluOpType.is_equal)
                        nc.tensor.matmul(agg_ps[:], lhsT=s_t[:],
                                         rhs=g_t[:, 0 if no_gather else ch, :],
                                         start=(ch == 0), stop=(ch == nch - 1))
                    # node-major agg -> SBUF -> transpose to feature-major
                    agg_nm = work.tile([P, P], dt.float32, tag="aggnm")
                    nc.scalar.copy(agg_nm[:], agg_ps[:])
                    aggT_ps = psT.tile([P, P], dt.float32, tag="aggT")
                    nc.tensor.transpose(aggT_ps[:], agg_nm[:], ident_t[:])
                    z1in = work.tile([P, P], dt.float32, tag="z1in")
                    nc.vector.tensor_add(z1in[:], h_loc[:, t * P:(t + 1) * P],
                                         aggT_ps[:])
                    # MLP (feature-major, weights stationary)
                    mp1 = psM.tile([P, P], dt.float32, tag="mp")
                    nc.tensor.matmul(mp1[:], lhsT=w1c, rhs=z1in[:],
                                     start=True, stop=True)
                    z1 = work.tile([P, P], dt.float32, tag="z1")
                    nc.scalar.activation(z1[:], mp1[:],
                                         mybir.ActivationFunctionType.Relu,
                                         bias=b1c)
                    mp2 = psM.tile([P, P], dt.float32, tag="mp")
                    nc.tensor.matmul(mp2[:], lhsT=w2c, rhs=z1[:],
                                     start=True, stop=True)
                    # z_pre overwrites h_loc tile in place
                    nc.scalar.activation(h_loc[:, t * P:(t + 1) * P], mp2[:],
                                         mybir.ActivationFunctionType.Relu,
                                         bias=b2c)
                    # zero phantom (unfilled + pad) seat columns, then stats
                    msk = spool.tile([P, P], dt.float32, tag="S")
                    nc.vector.tensor_scalar(
                        out=msk[:], in0=iota_t[:],
                        scalar1=filled_t[:, t:t + 1], scalar2=None,
                        op0=mybir.AluOpType.is_lt)
                    nc.vector.tensor_tensor(
                        out=h_loc[:, t * P:(t + 1) * P],
                        in0=h_loc[:, t * P:(t + 1) * P], in1=msk[:],
                        op=mybir.AluOpType.mult)
                    seat_ap = h_loc[:, t * P:(t + 1) * P]
                    nc.vector.tensor_reduce(
                        out=ssum[:, t:t + 1], in_=seat_ap,
                        axis=mybir.AxisListType.X, op=mybir.AluOpType.add)
                    sqs = work.tile([P, P], dt.float32, tag="sqs")
                    nc.scalar.activation(sqs[:], seat_ap,
                                         mybir.ActivationFunctionType.Square,
                                         accum_out=ssq[:, t:t + 1])

                # ---- BN stats allreduce ----
                red = work.tile([P, 2], dt.float32, tag="red")
                nc.vector.tensor_reduce(out=red[:, 0:1], in_=ssum[:],
                                        axis=mybir.AxisListType.X,
                                        op=mybir.AluOpType.add)
                nc.vector.tensor_reduce(out=red[:, 1:2], in_=ssq[:],
                                        axis=mybir.AxisListType.X,
                                        op=mybir.AluOpType.add)
                nc.sync.dma_start(st_in[l].ap(), red[:])
                if not no_cc:
                    nc.gpsimd.collective_compute(
                        "AllReduce", mybir.AluOpType.add, replica_groups=groups,
                        ins=[st_in[l].ap().opt()], outs=[st_out[l].ap().opt()])
                stt = work.tile([P, 2], dt.float32, tag="stt")
                nc.sync.dma_start(stt[:], (st_in[l] if no_cc else st_out[l]).ap())
                # mean, var, scale a, shift c
                mean = work.tile([P, 1], dt.float32, tag="mean")
                nc.vector.tensor_scalar(out=mean[:], in0=stt[:, 0:1],
                                        scalar1=inv_n, scalar2=None,
                                        op0=mybir.AluOpType.mult)
                msq = work.tile([P, 1], dt.float32, tag="msq")
                nc.vector.tensor_tensor(out=msq[:], in0=mean[:], in1=mean[:],
                                        op=mybir.AluOpType.mult)
                var = work.tile([P, 1], dt.float32, tag="var")
                nc.vector.tensor_scalar(out=var[:], in0=stt[:, 1:2],
                                        scalar1=inv_n, scalar2=BN_EPS,
                                        op0=mybir.AluOpType.mult,
                                        op1=mybir.AluOpType.add)
                nc.vector.tensor_tensor(out=var[:], in0=var[:], in1=msq[:],
                                        op=mybir.AluOpType.subtract)
                sd = work.tile([P, 1], dt.float32, tag="sd")
                nc.scalar.activation(sd[:], var[:],
                                     mybir.ActivationFunctionType.Sqrt)
                inv = work.tile([P, 1], dt.float32, tag="inv")
                nc.vector.reciprocal(inv[:], sd[:])
                a_t = work.tile([P, 1], dt.float32, tag="a")
                nc.vector.tensor_tensor(out=a_t[:], in0=inv[:], in1=gac,
                                        op=mybir.AluOpType.mult)
                c_t = work.tile([P, 1], dt.float32, tag="c")
                nc.vector.tensor_tensor(out=c_t[:], in0=mean[:], in1=a_t[:],
                                        op=mybir.AluOpType.mult)
                nc.vector.tensor_tensor(out=c_t[:], in0=bec, in1=c_t[:],
                                        op=mybir.AluOpType.subtract)

                # ---- normalize in place, transpose, pool, store ----
                pool_ps = psP.tile([P, P], dt.float32, tag="pool")
                for t in range(T):
                    nc.scalar.activation(
                        h_loc[:, t * P:(t + 1) * P],
                        h_loc[:, t * P:(t + 1) * P],
                        mybir.ActivationFunctionType.Identity,
                        bias=c_t[:, 0:1], scale=a_t[:, 0:1])
                    zT_ps = psT.tile([P, P], dt.float32, tag="aggT")
                    nc.tensor.transpose(zT_ps[:], h_loc[:, t * P:(t + 1) * P],
                                        ident_t[:])
                    znm = work.tile([P, P], dt.float32, tag="znm")
                    nc.scalar.copy(znm[:], zT_ps[:])
                    if l < L - 1:
                        nc.sync.dma_start(zz[l].ap()[t * P:(t + 1) * P, :],
                                          znm[:])
                    sb_t = spool.tile([P, P], dt.float32, tag="S")
                    nc.vector.tensor_scalar(
                        out=sb_t[:], in0=iota_t[:],
                        scalar1=brel_t[:, t:t + 1], scalar2=None,
                        op0=mybir.AluOpType.is_equal)
                    nc.tensor.matmul(pool_ps[:], lhsT=sb_t[:], rhs=znm[:],
                                     start=(t == 0), stop=(t == T - 1),
                                     skip_group_check=True)
                poolsb = work.tile([P, P], dt.float32, tag="poolsb")
                nc.scalar.copy(poolsb[:], pool_ps[:])
                nc.sync.dma_start(pools_d.ap()[:, l * F:(l + 1) * F],
                                  poolsb[:])

                if l < L - 1 and not no_cc:
                    nc.gpsimd.collective_compute(
                        "AllGather", mybir.AluOpType.bypass,
                        replica_groups=groups,
                        ins=[zz[l].ap().opt()], outs=[hf[l].ap().opt()])

    nc.compile()
    return nc


def make_in_maps(per_core, x_full, inputs):
    W1 = np.asarray(inputs['W1'], np.float32)
    W2 = np.asarray(inputs['W2'], np.float32)
    b1 = np.asarray(inputs['b1'], np.float32)
    b2 = np.asarray(inputs['b2'], np.float32)
    gamma = np.asarray(inputs['gamma'], np.float32)
    beta = np.asarray(inputs['beta'], np.float32)
    w1_h = np.ascontiguousarray(np.concatenate([W1[i] for i in range(L)], 1))
    w2_h = np.ascontiguousarray(np.concatenate([W2[i] for i in range(L)], 1))
    bias_h = np.ascontiguousarray(
        np.concatenate([b1.T, b2.T, gamma.T, beta.T], 1))
    iota_h = np.tile(np.arange(P, dtype=np.float32), (P, 1))
    ident_h = np.eye(P, dtype=np.float32)
    in_maps = []
    for c in range(NCORES):
        pc = per_core[c]
        in_maps.append({
            "xT": pc["xT"], "xnm": pc["xnm"], "gidx": pc["gidx"],
            "dstrel": pc["dstrel"], "brel": pc["batch_rel"],
            "filled": pc["filled"],
            "w1": w1_h, "w2": w2_h, "bias": bias_h,
            "iotat": iota_h, "ident": ident_h,
        })
    return in_maps


def build_null_kernel(cfg, BCPT):
    """Same I/O signature as the real kernel, trivial body. Used to measure
    the non-compute overhead (upload/dispatch) of a run for timing deltas."""
    CAPB = BCPT * 128
    T = cfg.T
    dt = mybir.dt
    nc = bacc.Bacc("TRN2", target_bir_lowering=False, debug=False,
                   enable_asserts=True, num_devices=NCORES,
                   num_swdge_queues=4)
    xT_d = nc.dram_tensor("xT", [P, cfg.COLS], dt.float32,
                          kind="ExternalInput")
    nc.dram_tensor("xnm", [cfg.COLS, F], dt.float32, kind="ExternalInput")
    nc.dram_tensor("gidx", [P, T * 4 * (CAPB // 16)], dt.int16,
                   kind="ExternalInput")
    nc.dram_tensor("dstrel", [P, T * 4 * BCPT], dt.float32,
                   kind="ExternalInput")
    nc.dram_tensor("brel", [P, T], dt.float32, kind="ExternalInput")
    nc.dram_tensor("filled", [P, T], dt.float32, kind="ExternalInput")
    nc.dram_tensor("w1", [P, L * F], dt.float32, kind="ExternalInput")
    nc.dram_tensor("w2", [P, L * F], dt.float32, kind="ExternalInput")
    nc.dram_tensor("bias", [P, 4 * L], dt.float32, kind="ExternalInput")
    nc.dram_tensor("iotat", [P, P], dt.float32, kind="ExternalInput")
    nc.dram_tensor("ident", [P, P], dt.float32, kind="ExternalInput")
    pools_d = nc.dram_tensor("pools", [P, L * F], dt.float32,
                             kind="ExternalOutput")
    with tile.TileContext(nc) as tc:
        with tc.tile_pool(name="sb", bufs=1) as sb:
            t0 = sb.tile([P, L * F], dt.float32)
            nc.sync.dma_start(t0[:], xT_d.ap()[:, 0:L * F])
            nc.sync.dma_start(pools_d.ap(), t0[:])
    nc.compile()
    return nc


# ---------------- entry point ----------------
def kernel(x, edge_index, batch, W1, b1, W2, b2, gamma, beta):
    cfg = FULL
    x = np.asarray(x, np.float32)
    edge_index = np.asarray(edge_index, np.int32)
    batch = np.asarray(batch, np.int32)
    W1 = np.asarray(W1, np.float32)
    b1 = np.asarray(b1, np.float32)
    W2 = np.asarray(W2, np.float32)
    b2 = np.asarray(b2, np.float32)
    gamma = np.asarray(gamma, np.float32)
    beta = np.asarray(beta, np.float32)

    x_full, per_core, meta = preprocess(cfg, x, edge_index, batch)
    nc = build_kernel(cfg, meta['BCPT'])
    in_maps = make_in_maps(per_core, x_full, dict(
        W1=W1, W2=W2, b1=b1, b2=b2, gamma=gamma, beta=beta))

    import time as _time
    last_exc = None
    for attempt in range(3):
        try:
            res = bass_utils.run_bass_kernel_spmd(
                nc, in_maps, core_ids=list(range(NCORES)))
            break
        except Exception as e:       # transient device wedge -> retry
            last_exc = e
            _time.sleep(20)
    else:
        raise last_exc

    out = np.zeros((NUM_GRAPHS, L * F), np.float32)
    for c in range(NCORES):
        pool_c = res.results[c]["pools"]          # [128, L*F]
        g0, sp = meta['g_bases'][c], meta['spans'][c]
        out[g0:g0 + sp] += pool_c[:sp]
    return out


if __name__ == "__main__":
    import reference
    inputs = reference.setup_inputs()
    inputs = {k: np.asarray(v) for k, v in inputs.items()}
    got = kernel(**inputs)
    print("kernel output shape:", got.shape)

